# revision 7
# baseline (speedup 1.0000x reference)
"""DANetHead Trainium2 kernel: 8-core SPMD (batch x row-half sharding).

Self-contained: hardcodes all shapes from the problem spec.

Per-core layout (core c: sample b=c//2, half h=c%2):
  P = [-1, 0..63, 64] (66 padded rows; -1/64 zero).
  feat local row L (0..65) holds padded row P[(L+32h) % 66] (cyclic rotation,
  so every core's attention/conv2 window is local rows 0..33 uniformly).
  window = local rows 0..33 (flat 0..2175); my output rows = 1..32.

Transfer-optimized: each core uploads only its own half of x (+1 halo row)
in fp16 inside a single packed blob; conv1 runs on the half, then the raw
conv1 outputs are pair-AllGathered on device and blended (per-core scalar
masks select the h=0/h=1 placement) into the full rotated feat layout.
Output is fp16. A custom PJRT runner avoids uploading donated zero output
buffers (the kernel writes every output element).
"""
import numpy as np

import jax
import jax.numpy as jnp
from jax.sharding import Mesh, PartitionSpec, NamedSharding
from jax.experimental.shard_map import shard_map

import concourse.bass as bass
import concourse.tile as tile
from concourse import bacc, mybir
from concourse.bass2jax import (_bass_exec_p, install_neuronx_cc_hook,
                                partition_id_tensor)

F32 = mybir.dt.float32
F32R = mybir.dt.float32r
F16 = mybir.dt.float16
AF = mybir.ActivationFunctionType
ALU = mybir.AluOpType

B, CIN, H, W = 4, 256, 64, 64
CI, CQ, CO = 64, 8, 256
NCORES = 8
LR = 66                  # local feat rows
NP = LR * W              # 4224
NJT = NP // 128          # 33 j-tiles
WIN = 34 * W             # 2176
MY = 32 * W              # 2048
NTAPS = 18               # 9 taps x 2 cin blocks
XR = 34                  # x rows per core (own 32 + halo)
# i chunks: CAM uses full window; PAM main loop uses ICM + tail
IC = [(0, 512), (512, 512), (1024, 512), (1536, 512), (2048, 128)]
ICM = [(0, 512), (512, 512), (1024, 512), (1536, 384), (1920, 256)]
N_STAT = 16384.0

# blob A (fp16) element offsets
OFF_XH = 0
OFF_W1 = OFF_XH + 128 * 2 * XR * 64      # 557056
OFF_W2A = OFF_W1 + 128 * NTAPS * CI      # 704512
OFF_W2B = OFF_W2A + 128 * 3 * CI         # 729088
OFF_QKV = OFF_W2B + 64 * 3 * CI          # 741376
OFF_W8 = OFF_QKV + 65 * 80               # 746576
OFF_ID = OFF_W8 + 65 * 256               # 763216
KA = OFF_ID + 128 * 128                  # 779600

# blob B (f32) element offsets
OFF_EB = 0
OFF_NM = OFF_EB + 2 * NP                 # 8448
OFF_HM = OFF_NM + 128 * NJT              # 12672
OFF_BG = OFF_HM + 64 * 2                 # 12800
OFF_CN = OFF_BG + 64 * 2                 # 12928
KB = OFF_CN + 2                          # 12930

PAD = [-1] + list(range(64)) + [64]


# ---------------------------------------------------------------- host prep
def _shared_blobs():
    """(blobA weight tail [KA-OFF_W1] fp16, blobB per-h variants [2, KB] f32)."""
    return None


def _prep_core_inputs(x, w1, bn_g, bn_b, wq, bq, wk, bk, wv, bv,
                      gamma_pam, gamma_cam, w2, w8, b8):
    f = np.float32
    # ---- shared weight tail of blob A (fp16)
    w1s = np.zeros((128, NTAPS, CI), f)
    for dy in range(3):
        for dx in range(3):
            for cb in range(2):
                s = (dy * 3 + dx) * 2 + cb
                w1s[:, s, :] = w1[:, cb * 128:(cb + 1) * 128, dy, dx].T
    w2a = np.zeros((128, 3, CI), f)
    w2b = np.zeros((64, 3, CI), f)
    for dx in range(3):
        w2a[:64, dx, :] = w2[:, :, 0, dx].T
        w2a[64:, dx, :] = w2[:, :, 1, dx].T
        w2b[:, dx, :] = w2[:, :, 2, dx].T
    wqkv = np.zeros((65, 80), f)
    wqkv[:64, 0:64] = wv[:, :, 0, 0].T
    wqkv[:64, 64:72] = wq[:, :, 0, 0].T
    wqkv[:64, 72:80] = wk[:, :, 0, 0].T
    wqkv[64, 0:64] = bv
    wqkv[64, 64:72] = bq
    wqkv[64, 72:80] = bk
    w8s = np.zeros((65, 2, 128), f)
    for blk in range(2):
        w8s[:64, blk, :] = w8[blk * 128:(blk + 1) * 128, :, 0, 0].T
        w8s[64, blk, :] = b8[blk * 128:(blk + 1) * 128]
    wtail = np.concatenate([
        w1s.ravel(), w2a.ravel(), w2b.ravel(), wqkv.ravel(), w8s.ravel(),
        np.eye(128, dtype=f).ravel()]).astype(np.float16)

    # ---- blob B per-h variants (f32)
    bngb = np.stack([bn_g, bn_b], 1).astype(f)
    consts = np.array([float(gamma_pam[0]), float(gamma_cam[0])], f)
    hbv = np.zeros((2, KB), f)
    for h in range(2):
        centers = [PAD[(L + 32 * h) % 66] for L in range(LR)]
        realp = np.repeat(np.array([0 <= g <= 63 for g in centers]), W)
        ebias = np.concatenate([np.where(realp, 0.0, -1000.0).astype(f),
                                np.ones(NP, f)])
        nmask = np.where(realp, 1.0, 0.0).astype(f).reshape(NJT, 128).T
        hmask = np.zeros((64, 2), f)
        hmask[:, 0] = 0.0 if h == 0 else 1.0
        hmask[:, 1] = 0.0 if h == 1 else 1.0
        hbv[h] = np.concatenate([ebias, nmask.ravel(), hmask.ravel(),
                                 bngb.ravel(), consts])

    # ---- per-core blob A: xh [128, 2, 34, 64] fp16 + shared weight tail
    x16 = np.asarray(x, np.float16)
    ha = np.empty((NCORES, KA), np.float16)
    ha[:, OFF_W1:] = wtail
    hb = np.empty((NCORES, KB), f)
    for c in range(NCORES):
        b, h = divmod(c, 2)
        xh = np.zeros((128, 2, XR, 64), np.float16)
        if h == 0:
            # rows r=1..33 <- image rows 0..32 (r=0 is the zero pad row)
            xh[:, 0, 1:34, :] = x16[b, :128, 0:33, :]
            xh[:, 1, 1:34, :] = x16[b, 128:, 0:33, :]
        else:
            # rows r=0..32 <- image rows 31..63 (r=33 is the zero pad row)
            xh[:, 0, 0:33, :] = x16[b, :128, 31:64, :]
            xh[:, 1, 0:33, :] = x16[b, 128:, 31:64, :]
        ha[c, :OFF_W1] = xh.ravel()
        hb[c] = hbv[h]
    return ha, hb


# ---------------------------------------------------------------- bass build
def _build():
    nc = bacc.Bacc()
    ha = nc.declare_dram_parameter("ha", [1, KA], F16, isOutput=False)
    hb = nc.declare_dram_parameter("hb", [1, KB], F32R, isOutput=False)
    out = nc.declare_dram_parameter("out", [NCORES * 256, MY], F16,
                                   isOutput=True)

    def hap(off, ap):
        return bass.AP(tensor=ha, offset=off, ap=ap)

    def hbp(off, ap):
        return bass.AP(tensor=hb, offset=off, ap=ap)

    with tile.TileContext(nc) as tc:
        with tc.tile_pool(name="big", bufs=1) as big, \
             tc.tile_pool(name="wt", bufs=1) as wt, \
             tc.tile_pool(name="sm", bufs=2) as sm, \
             tc.tile_pool(name="et", bufs=2) as etp, \
             tc.tile_pool(name="ps", bufs=2, space="PSUM") as ps, \
             tc.tile_pool(name="pt", bufs=2, space="PSUM") as ptp, \
             tc.tile_pool(name="mc", bufs=2, space="PSUM") as mcp, \
             tc.tile_pool(name="dram", bufs=1, space="DRAM") as dram:

            # ---- persistent sbuf tensors
            feat = big.tile([65, NP], F32R, tag="feat")   # y1 then feat1(+ones)
            qkv = big.tile([80, NP], F32R, tag="qkv")
            qr = big.tile([128, WIN], F32R, tag="qr")
            kr4 = big.tile([128, 9, 128], F32R, tag="kr4")
            vT = big.tile([128, NJT, 65], F32R, tag="vT")
            fT = big.tile([128, NJT, CI], F32R, tag="fT")
            sabuf = big.tile([128, 34, LR], F32R, tag="sabuf")
            scbuf = big.tile([128, 34, LR], F32R, tag="scbuf")
            y2a = big.tile([64, MY], F32, tag="y2a")
            y2b = big.tile([64, MY], F32, tag="y2b")
            fsum = big.tile([65, MY], F32R, tag="fsum")
            pacc = big.tile([65, WIN], F32, tag="pacc")   # pam accumulator
            xh = big.tile([128, 2, XR, LR], F16, tag="xh")

            # ---- weights / consts in sbuf
            w1t = wt.tile([128, NTAPS, CI], F16, tag="w1t")
            wqkvt = wt.tile([65, 80], F32R, tag="wqkvt")
            w2at = wt.tile([128, 3 * CI], F32R, tag="w2at")
            w2bt = wt.tile([64, 3 * CI], F32R, tag="w2bt")
            w8t = wt.tile([65, 256], F32R, tag="w8t")
            bngbt = wt.tile([64, 2], F32, tag="bngbt")
            nmt = wt.tile([128, NJT], F32, tag="nmt")
            hmt = wt.tile([64, 2], F32, tag="hmt")
            cst = wt.tile([1, 2], F32, tag="cst")
            gcam = wt.tile([64, 1], F32, tag="gcam")
            epst = wt.tile([64, 1], F32, tag="epst")
            nc.vector.memset(epst, 1e-5)
            idt = wt.tile([128, 128], F32R, tag="idt")
            # fp16 staging for converted weights
            wq16 = wt.tile([65, 80], F16, tag="wq16")
            w2a16 = wt.tile([128, 3 * CI], F16, tag="w2a16")
            w2b16 = wt.tile([64, 3 * CI], F16, tag="w2b16")
            w816 = wt.tile([65, 256], F16, tag="w816")
            id16 = wt.tile([128, 128], F16, tag="id16")

            nc.sync.dma_start(out=w1t, in_=hap(OFF_W1, [[NTAPS * CI, 128],
                                                        [CI, NTAPS], [1, CI]]))
            nc.sync.dma_start(out=w2a16, in_=hap(OFF_W2A, [[192, 128], [1, 192]]))
            nc.sync.dma_start(out=w2b16, in_=hap(OFF_W2B, [[192, 64], [1, 192]]))
            nc.sync.dma_start(out=wq16, in_=hap(OFF_QKV, [[80, 65], [1, 80]]))
            nc.sync.dma_start(out=w816, in_=hap(OFF_W8, [[256, 65], [1, 256]]))
            nc.sync.dma_start(out=id16, in_=hap(OFF_ID, [[128, 128], [1, 128]]))
            nc.vector.tensor_copy(w2at, w2a16)
            nc.vector.tensor_copy(w2bt, w2b16)
            nc.vector.tensor_copy(wqkvt, wq16)
            nc.vector.tensor_copy(w8t, w816)
            nc.vector.tensor_copy(idt, id16)

            nc.sync.dma_start(out=bngbt[:, :].bitcast(F32R), in_=hbp(OFF_BG, [[2, 64], [1, 2]]))
            nc.sync.dma_start(out=nmt[:, :].bitcast(F32R), in_=hbp(OFF_NM, [[NJT, 128], [1, NJT]]))
            nc.sync.dma_start(out=hmt[:, :].bitcast(F32R), in_=hbp(OFF_HM, [[2, 64], [1, 2]]))
            nc.sync.dma_start(out=cst[:, :].bitcast(F32R), in_=hbp(OFF_CN, [[2, 1], [1, 2]]))
            nc.gpsimd.dma_start(out=gcam, in_=hbp(OFF_CN + 1, [[0, 64], [1, 1]]))

            nc.gpsimd.memset(feat[64:65, :].bitcast(F32), 1.0)
            nc.gpsimd.memset(fsum[64:65, :].bitcast(F32), 1.0)
            nc.gpsimd.memset(kr4[:, :, :].bitcast(F32), 0.0)
            nc.gpsimd.memset(vT[:, :, 64:65].bitcast(F32), 1.0)
            for bf in (sabuf, scbuf):
                nc.gpsimd.memset(bf[0:64, :, 0:1].bitcast(F32), 0.0)
                nc.gpsimd.memset(bf[0:64, :, 65:66].bitcast(F32), 0.0)

            # ---- x half: zero pad cols, DMA real cols
            nc.gpsimd.memset(xh[:, :, :, :], 0.0)
            nc.sync.dma_start(
                out=xh[:, :, :, 1:65],
                in_=hap(OFF_XH, [[2 * XR * 64, 128], [XR * 64, 2],
                                 [64, XR], [1, 64]]))

            # ---- conv1 own half -> feat cols 64:2112 (raw y1) + stats
            stats1 = sm.tile([64, 4, 6], F32, tag="stats1")
            for T in range(4):
                pst = mcp.tile([64, 512], F32, tag="mc", name=f"c1ps{T}")
                for s in range(NTAPS):
                    tap, cb = divmod(s, 2)
                    dy, dx = divmod(tap, 3)
                    rhs = xh[:, cb, 8 * T + dy:8 * T + dy + 8, dx:dx + 64]
                    nc.tensor.matmul(pst, w1t[:, s, :], rhs,
                                     start=(s == 0), stop=(s == NTAPS - 1))
                nc.vector.bn_stats(stats1[:, T, :], pst)
                nc.vector.tensor_copy(feat[0:64, 64 + 512 * T:576 + 512 * T], pst)
            mv1 = sm.tile([64, 2], F32, tag="mv1")
            nc.vector.bn_aggr(mv1, stats1[:, :, :])

            # ---- pair-exchange raw y1, blend partner rows into feat
            y1d = dram.tile([64, MY], F32, tag="y1d")
            y1g = dram.tile([128, MY], F32, tag="y1g")
            nc.sync.dma_start(out=y1d[:, :], in_=feat[0:64, 64:2112].bitcast(F32))
            nc.gpsimd.collective_compute(
                "AllGather", ALU.bypass,
                replica_groups=[[0, 1], [2, 3], [4, 5], [6, 7]],
                ins=[y1d.opt()], outs=[y1g.opt()])
            # stage X: placement for h=0 receivers; stage Y: for h=1 receivers.
            # stage col s maps to feat col 2112+s (s<2112) or s-2112 (s>=2112).
            stX = sm.tile([64, WIN], F32, tag="stX")
            stY = sm.tile([64, WIN], F32, tag="stY")
            nc.vector.memset(stX[:, 2048:2176], 0.0)
            nc.vector.memset(stY[:, 0:128], 0.0)
            nc.sync.dma_start(out=stX[:, 0:2048], in_=y1g[64:128, :])
            nc.sync.dma_start(out=stY[:, 128:2112], in_=y1g[0:64, 0:1984])
            nc.sync.dma_start(out=stY[:, 2112:2176], in_=y1g[0:64, 1984:2048])
            nc.vector.tensor_scalar_mul(stX, stX, hmt[:, 1:2])   # keep iff h==0
            nc.vector.tensor_scalar_mul(stY, stY, hmt[:, 0:1])   # keep iff h==1
            nc.vector.tensor_tensor(feat[0:64, 2112:4224],
                                    stX[:, 0:2112], stY[:, 0:2112], ALU.add)
            nc.vector.tensor_tensor(feat[0:64, 0:64],
                                    stX[:, 2112:2176], stY[:, 2112:2176],
                                    ALU.add)

            def bn_coeffs(gl, tag):
                """gl [64,2] = (sum, sumsq) -> (scale, shift) [64,1] f32."""
                mean = sm.tile([64, 1], F32, tag=tag + "m", name=tag + "m")
                var = sm.tile([64, 1], F32, tag=tag + "v", name=tag + "v")
                scl = sm.tile([64, 1], F32, tag=tag + "s", name=tag + "s")
                sh = sm.tile([64, 1], F32, tag=tag + "h", name=tag + "h")
                nc.vector.tensor_scalar_mul(mean, gl[:, 0:1], 1.0 / N_STAT)
                nc.vector.tensor_scalar_mul(var, gl[:, 1:2], 1.0 / N_STAT)
                nc.vector.tensor_tensor(scl, mean, mean, ALU.mult)
                nc.vector.tensor_tensor(var, var, scl, ALU.subtract)
                nc.scalar.activation(var, var, AF.Sqrt, bias=epst, scale=1.0)
                nc.vector.reciprocal(var, var)
                nc.vector.tensor_tensor(scl, bngbt[:, 0:1], var, ALU.mult)
                nc.vector.tensor_tensor(sh, mean, scl, ALU.mult)
                nc.vector.tensor_tensor(sh, bngbt[:, 1:2], sh, ALU.subtract)
                return scl, sh

            def stat_ar(mv, tag):
                """partial (mean,var over MY) -> AllReduce -> (sum,sumsq)."""
                ars = sm.tile([64, 2], F32, tag=tag + "s", name=tag + "s")
                t_t = sm.tile([64, 1], F32, tag=tag + "t", name=tag + "t")
                nc.vector.tensor_scalar_mul(ars[:, 0:1], mv[:, 0:1], float(MY))
                nc.vector.tensor_tensor(t_t, mv[:, 0:1], mv[:, 0:1], ALU.mult)
                nc.vector.tensor_tensor(t_t, mv[:, 1:2], t_t, ALU.add)
                nc.vector.tensor_scalar_mul(ars[:, 1:2], t_t, float(MY))
                a_in = dram.tile([64, 2], F32, tag=tag + "_in", name=tag + "_in")
                a_out = dram.tile([64, 2], F32, tag=tag + "_out",
                                  name=tag + "_out")
                nc.sync.dma_start(out=a_in[:, :], in_=ars)
                nc.gpsimd.collective_compute(
                    "AllReduce", ALU.add,
                    replica_groups=[list(range(NCORES))],
                    ins=[a_in.opt()], outs=[a_out.opt()])
                gl = sm.tile([64, 2], F32, tag=tag + "g", name=tag + "g")
                nc.sync.dma_start(out=gl, in_=a_out[:, :])
                return gl

            # AR1: bn1 stats -> relu(bn(y1)) over all 66 local rows
            gl1 = stat_ar(mv1, "ar1")
            sc1, sh1 = bn_coeffs(gl1, "bn1")
            for c0 in range(0, NP, 1056):
                sl = feat[0:64, c0:c0 + 1056]
                nc.scalar.activation(sl, sl, AF.Relu, bias=sh1, scale=sc1)

            # ---- qkv
            qkvtiles = [(t * 512, 512) for t in range(8)] + [(4096, 128)]
            for ti, (c0, cw) in enumerate(qkvtiles):
                qps = mcp.tile([80, cw], F32, tag="mc", name="qps")
                nc.tensor.matmul(qps, wqkvt, feat[:, c0:c0 + cw],
                                 start=True, stop=True)
                nc.vector.tensor_copy(qkv[:, c0:c0 + cw], qps)
            # qr: q replicated at partition groups; row 32g+8 = ones
            # (pairs with the ebias row in kr4 -> energy gets +ebias[j])
            for g in range(4):
                nc.sync.dma_start(out=qr[32 * g:32 * g + 8, :],
                                  in_=qkv[64:72, 0:WIN])
            for g in range(4):
                nc.sync.dma_start(
                    out=qr[32 * g + 8:32 * g + 9, :],
                    in_=hbp(OFF_EB + NP, [[NP, 1], [1, WIN]]))
            # kr4: k repartitioned per j-group; row 8 of each 32-block holds
            # the exp masking bias for that j-tile
            kbounce = dram.tile([8, NP], F32R, tag="kbounce", name="kbounce")
            nc.sync.dma_start(out=kbounce[:, :], in_=qkv[72:80, :])
            for u in range(4):
                ksrc = bass.AP(tensor=kbounce.tensor,
                               offset=kbounce.offset + u * 128,
                               ap=[[NP, 8], [512, 8], [1, 128]])
                nc.sync.dma_start(out=kr4[32 * u:32 * u + 8, 0:8, :], in_=ksrc)
                nc.sync.dma_start(
                    out=kr4[32 * u + 8:32 * u + 9, 0:8, :],
                    in_=hbp(OFF_EB + u * 128, [[512, 8], [1, 128]]))
            nc.sync.dma_start(out=kr4[0:8, 8, :], in_=kbounce[:, 4096:4224])
            nc.sync.dma_start(out=kr4[8:9, 8, :],
                              in_=hbp(OFF_EB + 4096, [[NP, 1], [1, 128]]))

            # ---- vT transpose (+ones col), 4 per psum bank
            for j0 in range(0, 32, 4):
                tp = mcp.tile([128, 4, 64], F32R, tag="mc", name=f"vtp{j0}")
                for k in range(4):
                    jt = j0 + k
                    nc.tensor.transpose(
                        tp[:, k, :],
                        qkv[0:64, jt * 128:(jt + 1) * 128],
                        idt[0:64, 0:64])
                nc.vector.tensor_copy(vT[:, j0:j0 + 4, 0:64], tp)
            tpl = mcp.tile([128, 64], F32R, tag="mc", name="vtpl")
            nc.tensor.transpose(tpl, qkv[0:64, 32 * 128:33 * 128],
                                idt[0:64, 0:64])
            nc.vector.tensor_copy(vT[:, 32, 0:64], tpl)

            # ================= interleaved attention + CAM emission ========
            def pam_pair(jg0, chunk_cb=None):
                """Emit energy/exp/pam for j-groups jg0, jg0+1 (or lone 8)."""
                jgs = [jg0] if jg0 == 8 else [jg0, jg0 + 1]
                for ici, (i0, iw) in enumerate(ICM):
                    pt = ptp.tile([65, iw], F32, tag="pt", name="pt")
                    nmm = sum(4 if j < 8 else 1 for j in jgs)
                    k = 0
                    for jg in jgs:
                        nu2 = 2 if jg < 8 else 1
                        for p in range(2 if jg < 8 else 1):
                            et_ps = ps.tile([128, 2, 512], F32, tag="ps",
                                            name="et_ps")
                            for u2 in range(nu2):
                                u = 2 * p + u2
                                nc.tensor.matmul(
                                    et_ps[:, u2, 0:iw],
                                    kr4[32 * u:32 * u + 32, jg, :],
                                    qr[32 * u:32 * u + 32, i0:i0 + iw],
                                    start=True, stop=True,
                                    tile_position=(32 * u, 0))
                            eT = etp.tile([128, 2, 512], F32R, tag="et",
                                          bufs=2, name="eT")
                            if nu2 == 2:
                                nc.scalar.activation(eT[:, :, 0:iw],
                                                     et_ps[:, :, 0:iw],
                                                     AF.Exp, bias=0.0,
                                                     scale=1.0)
                            else:
                                nc.scalar.activation(eT[:, 0, 0:iw],
                                                     et_ps[:, 0, 0:iw],
                                                     AF.Exp, bias=0.0,
                                                     scale=1.0)
                            for u2 in range(nu2):
                                jt = 4 * jg + 2 * p + u2
                                nc.tensor.matmul(pt, vT[:, jt, :],
                                                 eT[:, u2, 0:iw],
                                                 start=(k == 0),
                                                 stop=(k == nmm - 1))
                                k += 1
                    if jg0 == 0:
                        nc.vector.tensor_copy(pacc[:, i0:i0 + iw], pt)
                    else:
                        nc.vector.tensor_tensor(pacc[:, i0:i0 + iw],
                                                pacc[:, i0:i0 + iw], pt,
                                                ALU.add)
                    if chunk_cb is not None:
                        chunk_cb(ici, i0, iw)

            pam_pair(0)
            # fT transposes (CAM input), masked
            for jt in range(NJT):
                tp = mcp.tile([128, 64], F32R, tag="mc", name=f"ftp{jt}")
                nc.tensor.transpose(tp, feat[0:64, jt * 128:(jt + 1) * 128],
                                    idt[0:64, 0:64])
                nc.vector.tensor_scalar_mul(fT[:, jt, :], tp, nmt[:, jt:jt + 1])

            pam_pair(2)
            # CAM: ce (chunked), softmax, cattnT
            ce_sb = sm.tile([64, 64], F32, tag="ce_sb")
            for ci_, (j0, nj) in enumerate([(0, 9), (9, 8), (17, 8), (25, 8)]):
                ce_ps = mcp.tile([64, 64], F32, tag="mc", name=f"ce{ci_}")
                for k in range(nj):
                    jt = j0 + k
                    nc.tensor.matmul(ce_ps, fT[:, jt, :], fT[:, jt, :],
                                     start=(k == 0), stop=(k == nj - 1))
                if ci_ == 0:
                    nc.vector.tensor_copy(ce_sb, ce_ps)
                else:
                    nc.vector.tensor_tensor(ce_sb, ce_sb, ce_ps, ALU.add)
            rmin = sm.tile([64, 1], F32, tag="rmin")
            nc.vector.tensor_reduce(rmin, ce_sb, mybir.AxisListType.X, ALU.min)
            cu = sm.tile([64, 64], F32, tag="cu")
            nc.scalar.activation(cu, ce_sb, AF.Exp, bias=rmin, scale=-1.0)
            rs = sm.tile([64, 1], F32, tag="rs")
            nc.vector.tensor_reduce(rs, cu, mybir.AxisListType.X, ALU.add)
            nc.vector.reciprocal(rs, rs)
            cattn = sm.tile([64, 64], F32R, tag="cattn")
            nc.vector.tensor_scalar_mul(cattn, cu, rs)
            ctp = mcp.tile([64, 64], F32R, tag="mc", name="ctp")
            nc.tensor.transpose(ctp, cattn, idt[0:64, 0:64])
            cattnT = sm.tile([64, 64], F32R, tag="cattnT")
            nc.vector.tensor_copy(cattnT, ctp)

            pam_pair(4)
            # CAM apply + scbuf
            for (i0, iw) in IC:
                cam_ps = mcp.tile([64, iw], F32, tag="mc", name="cam_ps")
                nc.tensor.matmul(cam_ps, cattnT, feat[0:64, i0:i0 + iw],
                                 start=True, stop=True)
                tmpc = etp.tile([64, iw], F32R, tag="camt", bufs=3,
                                name="tmpc")
                nc.vector.tensor_scalar_mul(tmpc, cam_ps, gcam)
                r0, nr = i0 // W, iw // W
                nc.vector.tensor_tensor(
                    scbuf[0:64, r0:r0 + nr, 1:65],
                    tmpc[:, :].rearrange("p (r c) -> p r c", c=W),
                    feat[0:64, i0:i0 + iw].rearrange("p (r c) -> p r c", c=W),
                    ALU.add)
            nc.vector.tensor_scalar_mul(scbuf[0:64, 0, 1:65],
                                        scbuf[0:64, 0, 1:65], hmt[:, 0:1])
            nc.vector.tensor_scalar_mul(scbuf[0:64, 33, 1:65],
                                        scbuf[0:64, 33, 1:65], hmt[:, 1:2])
            for (a, b) in [(0, 9), (9, 17), (17, 25), (25, 33)]:
                nc.gpsimd.tensor_copy(scbuf[64:128, a:b, :],
                                      scbuf[0:64, a + 1:b + 1, :])

            def conv2(buf, y2sb, sttag):
                st = sm.tile([64, 4, 6], F32, tag=sttag, name=sttag)
                for T in range(4):
                    r0 = 1 + 8 * T
                    yps = mcp.tile([64, 512], F32, tag="mc", name="yps")
                    for dxi in range(3):
                        rhs1 = buf[:, r0 - 1:r0 + 7, dxi:dxi + 64]
                        nc.tensor.matmul(yps, w2at[:, dxi * 64:(dxi + 1) * 64],
                                         rhs1, start=(dxi == 0), stop=False)
                        rhs2 = buf[0:64, r0 + 1:r0 + 9, dxi:dxi + 64]
                        nc.tensor.matmul(yps, w2bt[:, dxi * 64:(dxi + 1) * 64],
                                         rhs2, start=False, stop=(dxi == 2))
                    nc.vector.bn_stats(st[:, T, :], yps)
                    nc.vector.tensor_copy(y2sb[:, T * 512:(T + 1) * 512], yps)
                mv = sm.tile([64, 2], F32, tag=sttag + "mv", name=sttag + "mv")
                nc.vector.bn_aggr(mv, st[:, :, :])
                return mv

            pam_pair(6)
            # conv2 on CAM branch + its stats AR (hidden under attention)
            mvb = conv2(scbuf, y2b, "stb")
            glb = stat_ar(mvb, "arb")
            scb, shb = bn_coeffs(glb, "bnb")
            rb = big.tile([64, MY], F32R, tag="rb")
            nc.scalar.activation(rb, y2b, AF.Relu, bias=shb, scale=scb)

            # ---- pam normalize (r = gamma_pam / s), sa = pam_u*r + feat1
            def pam_div(src, i0, iw, sfx):
                r32 = sm.tile([1, iw], F32, tag="r32", name="r32" + sfx)
                nc.vector.reciprocal(r32, src[64:65, :])
                rr = sm.tile([1, iw], F32R, tag="rr", name="rr" + sfx)
                nc.vector.tensor_scalar_mul(rr, r32, cst[0:1, 0:1])
                rbc = etp.tile([64, iw], F32R, tag="camt", bufs=3,
                               name="rbc" + sfx)
                nc.gpsimd.partition_broadcast(rbc, rr)
                tmpa = etp.tile([64, iw], F32R, tag="camt", bufs=3,
                                name="tmpa" + sfx)
                nc.vector.tensor_tensor(tmpa, src[0:64, :], rbc, ALU.mult)
                r0, nr = i0 // W, iw // W
                nc.vector.tensor_tensor(
                    sabuf[0:64, r0:r0 + nr, 1:65],
                    tmpa[:, :].rearrange("p (r c) -> p r c", c=W),
                    feat[0:64, i0:i0 + iw].rearrange("p (r c) -> p r c", c=W),
                    ALU.add)

            pam_pair(8, chunk_cb=lambda ici, i0, iw: pam_div(
                pacc[:, i0:i0 + iw], i0, iw, str(ici)))
            nc.vector.tensor_scalar_mul(sabuf[0:64, 0, 1:65],
                                        sabuf[0:64, 0, 1:65], hmt[:, 0:1])
            nc.vector.tensor_scalar_mul(sabuf[0:64, 33, 1:65],
                                        sabuf[0:64, 33, 1:65], hmt[:, 1:2])
            for (a, b) in [(0, 9), (9, 17), (17, 25), (25, 33)]:
                nc.gpsimd.tensor_copy(sabuf[64:128, a:b, :],
                                      sabuf[0:64, a + 1:b + 1, :])

            mva = conv2(sabuf, y2a, "sta")
            gla = stat_ar(mva, "ara")
            sca, sha = bn_coeffs(gla, "bna")

            ogin = dram.tile([256, MY], F16, tag="ogin")
            # ---- relu + sum + conv8, pipelined per 512 chunk
            for T in range(4):
                sl = slice(T * 512, (T + 1) * 512)
                ra = etp.tile([64, 512], F32R, tag="camt", bufs=3,
                              name=f"ra{T}")
                nc.scalar.activation(ra, y2a[:, sl], AF.Relu,
                                     bias=sha, scale=sca)
                nc.vector.tensor_tensor(fsum[0:64, sl], ra, rb[:, sl], ALU.add)
                for blk in range(2):
                    ops_ = mcp.tile([128, 512], F32, tag="mc", name="ops")
                    nc.tensor.matmul(ops_, w8t[:, blk * 128:(blk + 1) * 128],
                                     fsum[:, sl], start=True, stop=True)
                    osb = etp.tile([128, 512], F16, tag="osb", bufs=3,
                                   name="osb")
                    nc.vector.tensor_copy(osb, ops_)
                    nc.sync.dma_start(out=ogin[blk * 128:(blk + 1) * 128, sl],
                                      in_=osb)
            # gather all cores' outputs onto every core; the host fetches a
            # single shard (one D2H round trip instead of eight)
            ogout = dram.tile([NCORES * 256, MY], F16, tag="ogout")
            nc.gpsimd.collective_compute(
                "AllGather", ALU.bypass,
                replica_groups=[list(range(NCORES))],
                ins=[ogin.opt()], outs=[ogout.opt()])
            nc.sync.dma_start(out=out[:, :], in_=ogout[:, :])
    nc.finalize()
    return nc


# ---------------------------------------------------------------- runner
class _Runner:
    def __init__(self, nc, n_cores=NCORES):
        install_neuronx_cc_hook()
        self.nc = nc
        self.n_cores = n_cores
        in_names, out_names, out_avals, zero_shapes = [], [], [], []
        pname = nc.partition_id_tensor.name if nc.partition_id_tensor else None
        for alloc in nc.m.functions[0].allocations:
            if not isinstance(alloc, mybir.MemoryLocationSet):
                continue
            name = alloc.memorylocations[0].name
            if alloc.kind == "ExternalInput":
                if name != pname:
                    in_names.append(name)
            elif alloc.kind == "ExternalOutput":
                out_names.append(name)
                shape = tuple(alloc.tensor_shape)
                dtype = mybir.dt.np(alloc.dtype)
                out_avals.append(jax.core.ShapedArray(shape, dtype))
                zero_shapes.append((shape, dtype))
        self.n_params = len(in_names)
        self.in_names = in_names + out_names
        if pname is not None:
            self.in_names.append(pname)
        self.out_names = out_names

        devices = jax.devices()[:n_cores]
        self.mesh = Mesh(np.asarray(devices), ("core",))
        self.sharding = NamedSharding(self.mesh, PartitionSpec("core"))

        in_names_t = tuple(self.in_names)
        out_names_t = tuple(out_names)
        out_avals_t = tuple(out_avals)
        has_pid = pname is not None

        def _body(*args):
            operands = list(args)
            if has_pid:
                operands.append(partition_id_tensor())
            outs = _bass_exec_p.bind(
                *operands,
                out_avals=out_avals_t,
                in_names=in_names_t,
                out_names=out_names_t,
                lowering_input_output_aliases=(),
                sim_require_finite=True,
                sim_require_nnan=True,
                nc=nc,
            )
            return tuple(outs)

        n_args = self.n_params + len(out_names)
        self.fn = jax.jit(
            shard_map(_body, mesh=self.mesh,
                      in_specs=(PartitionSpec("core"),) * n_args,
                      out_specs=(PartitionSpec("core"),) * len(out_names),
                      check_rep=False),
            keep_unused=True,
        )
        # cached placeholder "output" operands: device-resident, never
        # donated, never transferred again. The kernel writes every output
        # element so their contents are irrelevant.
        self.placeholders = [
            jax.jit(lambda s=shape, d=dtype: jnp.zeros((n_cores * s[0],
                                                        *s[1:]), d),
                    out_shardings=self.sharding)()
            for shape, dtype in zero_shapes
        ]

    def __call__(self, *concat_inputs):
        dev_inputs = [jax.device_put(a, self.sharding) for a in concat_inputs]
        outs = self.fn(*dev_inputs, *self.placeholders)
        # every core holds the full gathered output; fetch shard 0 only
        return [np.asarray(o.addressable_shards[0].data) for o in outs]


_CACHE = {}


def kernel(**inputs):
    if "runner" not in _CACHE:
        _CACHE["runner"] = _Runner(_build())
    runner = _CACHE["runner"]
    ha, hb = _prep_core_inputs(
        np.asarray(inputs["x"], np.float32), np.asarray(inputs["w1"]),
        np.asarray(inputs["bn_g"]), np.asarray(inputs["bn_b"]),
        np.asarray(inputs["wq"]), np.asarray(inputs["bq"]),
        np.asarray(inputs["wk"]), np.asarray(inputs["bk"]),
        np.asarray(inputs["wv"]), np.asarray(inputs["bv"]),
        np.asarray(inputs["gamma_pam"]), np.asarray(inputs["gamma_cam"]),
        np.asarray(inputs["w2"]), np.asarray(inputs["w8"]),
        np.asarray(inputs["b8"]))
    res = runner(ha, hb)
    og = res[0].reshape(NCORES, CO, 32, W).astype(np.float32)
    out = np.empty((B, CO, H, W), np.float32)
    for c in range(NCORES):
        b, h = divmod(c, 2)
        out[b, :, 32 * h:32 * h + 32, :] = og[c]
    return out


# revision 9
# speedup vs baseline: 1.3837x; 1.3837x over previous
"""DANetHead Trainium2 kernel: 8-core SPMD (batch x row-half sharding).

Self-contained: hardcodes all shapes from the problem spec.

Per-core layout (core c: sample b=c//2, half h=c%2):
  P = [-1, 0..63, 64] (66 padded rows; -1/64 zero).
  feat local row L (0..65) holds padded row P[(L+32h) % 66] (cyclic rotation,
  so every core's attention/conv2 window is local rows 0..33 uniformly).
  window = local rows 0..33 (flat 0..2175); my output rows = 1..32.

Transfer-optimized: each core uploads only its own half of x (+1 halo row)
in fp16 inside a single packed blob; conv1 runs on the half, then the raw
conv1 outputs are pair-AllGathered on device and blended (per-core scalar
masks select the h=0/h=1 placement) into the full rotated feat layout.
Output is fp16. A custom PJRT runner avoids uploading donated zero output
buffers (the kernel writes every output element).
"""
import numpy as np

import jax
import jax.numpy as jnp
from jax.sharding import Mesh, PartitionSpec, NamedSharding
from jax.experimental.shard_map import shard_map

import concourse.bass as bass
import concourse.tile as tile
from concourse import bacc, mybir
from concourse.bass2jax import (_bass_exec_p, install_neuronx_cc_hook,
                                partition_id_tensor)

F32 = mybir.dt.float32
F32R = mybir.dt.float32r
F16 = mybir.dt.float16
AF = mybir.ActivationFunctionType
ALU = mybir.AluOpType

B, CIN, H, W = 4, 256, 64, 64
CI, CQ, CO = 64, 8, 256
NCORES = 8
LR = 66                  # local feat rows
NP = LR * W              # 4224
NJT = NP // 128          # 33 j-tiles
WIN = 34 * W             # 2176
MY = 32 * W              # 2048
NTAPS = 18               # 9 taps x 2 cin blocks
XR = 34                  # x rows per core (own 32 + halo)
# i chunks: CAM uses full window; PAM main loop uses ICM + tail
IC = [(0, 512), (512, 512), (1024, 512), (1536, 512), (2048, 128)]
ICM = [(0, 512), (512, 512), (1024, 512), (1536, 384), (1920, 256)]
N_STAT = 16384.0

# blob A (fp16) element offsets
OFF_XH = 0
OFF_W1 = OFF_XH + 128 * 2 * XR * 64      # 557056
OFF_W2A = OFF_W1 + 128 * NTAPS * CI      # 704512
OFF_W2B = OFF_W2A + 128 * 3 * CI         # 729088
OFF_QKV = OFF_W2B + 64 * 3 * CI          # 741376
OFF_ID = OFF_QKV + 65 * 80               # 746576
KA = OFF_ID + 128 * 128                  # 762960

# blob B (f32) element offsets
OFF_EB = 0
OFF_NM = OFF_EB + 2 * NP                 # 8448
OFF_HM = OFF_NM + 128 * NJT              # 12672
OFF_BG = OFF_HM + 64 * 2                 # 12800
OFF_CN = OFF_BG + 64 * 2                 # 12928
KB = OFF_CN + 2                          # 12930

PAD = [-1] + list(range(64)) + [64]


# ---------------------------------------------------------------- host prep
def _shared_blobs():
    """(blobA weight tail [KA-OFF_W1] fp16, blobB per-h variants [2, KB] f32)."""
    return None


def _prep_core_inputs(x, w1, bn_g, bn_b, wq, bq, wk, bk, wv, bv,
                      gamma_pam, gamma_cam, w2, w8, b8):
    f = np.float32
    # ---- shared weight tail of blob A (fp16)
    w1s = np.zeros((128, NTAPS, CI), f)
    for dy in range(3):
        for dx in range(3):
            for cb in range(2):
                s = (dy * 3 + dx) * 2 + cb
                w1s[:, s, :] = w1[:, cb * 128:(cb + 1) * 128, dy, dx].T
    w2a = np.zeros((128, 3, CI), f)
    w2b = np.zeros((64, 3, CI), f)
    for dx in range(3):
        w2a[:64, dx, :] = w2[:, :, 0, dx].T
        w2a[64:, dx, :] = w2[:, :, 1, dx].T
        w2b[:, dx, :] = w2[:, :, 2, dx].T
    wqkv = np.zeros((65, 80), f)
    wqkv[:64, 0:64] = wv[:, :, 0, 0].T
    wqkv[:64, 64:72] = wq[:, :, 0, 0].T
    wqkv[:64, 72:80] = wk[:, :, 0, 0].T
    wqkv[64, 0:64] = bv
    wqkv[64, 64:72] = bq
    wqkv[64, 72:80] = bk
    wtail = np.concatenate([
        w1s.ravel(), w2a.ravel(), w2b.ravel(), wqkv.ravel(),
        np.eye(128, dtype=f).ravel()]).astype(np.float16)

    # ---- blob B per-h variants (f32)
    bngb = np.stack([bn_g, bn_b], 1).astype(f)
    consts = np.array([float(gamma_pam[0]), float(gamma_cam[0])], f)
    hbv = np.zeros((2, KB), f)
    for h in range(2):
        centers = [PAD[(L + 32 * h) % 66] for L in range(LR)]
        realp = np.repeat(np.array([0 <= g <= 63 for g in centers]), W)
        ebias = np.concatenate([np.where(realp, 0.0, -1000.0).astype(f),
                                np.ones(NP, f)])
        nmask = np.where(realp, 1.0, 0.0).astype(f).reshape(NJT, 128).T
        hmask = np.zeros((64, 2), f)
        hmask[:, 0] = 0.0 if h == 0 else 1.0
        hmask[:, 1] = 0.0 if h == 1 else 1.0
        hbv[h] = np.concatenate([ebias, nmask.ravel(), hmask.ravel(),
                                 bngb.ravel(), consts])

    # ---- per-core blob A: xh [128, 2, 34, 64] fp16 + shared weight tail
    x16 = np.asarray(x, np.float16)
    ha = np.empty((NCORES, KA), np.float16)
    ha[:, OFF_W1:] = wtail
    hb = np.empty((NCORES, KB), f)
    for c in range(NCORES):
        b, h = divmod(c, 2)
        xh = np.zeros((128, 2, XR, 64), np.float16)
        if h == 0:
            # rows r=1..33 <- image rows 0..32 (r=0 is the zero pad row)
            xh[:, 0, 1:34, :] = x16[b, :128, 0:33, :]
            xh[:, 1, 1:34, :] = x16[b, 128:, 0:33, :]
        else:
            # rows r=0..32 <- image rows 31..63 (r=33 is the zero pad row)
            xh[:, 0, 0:33, :] = x16[b, :128, 31:64, :]
            xh[:, 1, 0:33, :] = x16[b, 128:, 31:64, :]
        ha[c, :OFF_W1] = xh.ravel()
        hb[c] = hbv[h]
    return ha, hb


# ---------------------------------------------------------------- bass build
def _build():
    nc = bacc.Bacc()
    ha = nc.declare_dram_parameter("ha", [1, KA], F16, isOutput=False)
    hb = nc.declare_dram_parameter("hb", [1, KB], F32R, isOutput=False)
    out = nc.declare_dram_parameter("out", [64, MY], F16, isOutput=True)

    def hap(off, ap):
        return bass.AP(tensor=ha, offset=off, ap=ap)

    def hbp(off, ap):
        return bass.AP(tensor=hb, offset=off, ap=ap)

    with tile.TileContext(nc) as tc:
        with tc.tile_pool(name="big", bufs=1) as big, \
             tc.tile_pool(name="wt", bufs=1) as wt, \
             tc.tile_pool(name="sm", bufs=2) as sm, \
             tc.tile_pool(name="et", bufs=2) as etp, \
             tc.tile_pool(name="ps", bufs=2, space="PSUM") as ps, \
             tc.tile_pool(name="pt", bufs=2, space="PSUM") as ptp, \
             tc.tile_pool(name="mc", bufs=2, space="PSUM") as mcp, \
             tc.tile_pool(name="dram", bufs=1, space="DRAM") as dram:

            # ---- persistent sbuf tensors
            feat = big.tile([65, NP], F32R, tag="feat")   # y1 then feat1(+ones)
            qkv = big.tile([80, NP], F32R, tag="qkv")
            qr = big.tile([128, WIN], F32R, tag="qr")
            kr4 = big.tile([128, 9, 128], F32R, tag="kr4")
            vT = big.tile([128, NJT, 65], F32R, tag="vT")
            fT = big.tile([128, NJT, CI], F32R, tag="fT")
            sabuf = big.tile([128, 34, LR], F32R, tag="sabuf")
            scbuf = big.tile([128, 34, LR], F32R, tag="scbuf")
            y2a = big.tile([64, MY], F32, tag="y2a")
            y2b = big.tile([64, MY], F32, tag="y2b")
            fs16 = big.tile([64, MY], F16, tag="fs16")
            pacc = big.tile([65, WIN], F32, tag="pacc")   # pam accumulator
            xh = big.tile([128, 2, XR, LR], F16, tag="xh")

            # ---- weights / consts in sbuf
            w1t = wt.tile([128, NTAPS, CI], F16, tag="w1t")
            wqkvt = wt.tile([65, 80], F32R, tag="wqkvt")
            w2at = wt.tile([128, 3 * CI], F32R, tag="w2at")
            w2bt = wt.tile([64, 3 * CI], F32R, tag="w2bt")
            bngbt = wt.tile([64, 2], F32, tag="bngbt")
            nmt = wt.tile([128, NJT], F32, tag="nmt")
            hmt = wt.tile([64, 2], F32, tag="hmt")
            cst = wt.tile([1, 2], F32, tag="cst")
            gcam = wt.tile([64, 1], F32, tag="gcam")
            epst = wt.tile([64, 1], F32, tag="epst")
            nc.vector.memset(epst, 1e-5)
            idt = wt.tile([128, 128], F32R, tag="idt")
            # fp16 staging for converted weights
            wq16 = wt.tile([65, 80], F16, tag="wq16")
            w2a16 = wt.tile([128, 3 * CI], F16, tag="w2a16")
            w2b16 = wt.tile([64, 3 * CI], F16, tag="w2b16")
            id16 = wt.tile([128, 128], F16, tag="id16")

            nc.sync.dma_start(out=w1t, in_=hap(OFF_W1, [[NTAPS * CI, 128],
                                                        [CI, NTAPS], [1, CI]]))
            nc.sync.dma_start(out=w2a16, in_=hap(OFF_W2A, [[192, 128], [1, 192]]))
            nc.sync.dma_start(out=w2b16, in_=hap(OFF_W2B, [[192, 64], [1, 192]]))
            nc.sync.dma_start(out=wq16, in_=hap(OFF_QKV, [[80, 65], [1, 80]]))
            nc.sync.dma_start(out=id16, in_=hap(OFF_ID, [[128, 128], [1, 128]]))
            nc.vector.tensor_copy(w2at, w2a16)
            nc.vector.tensor_copy(w2bt, w2b16)
            nc.vector.tensor_copy(wqkvt, wq16)
            nc.vector.tensor_copy(idt, id16)

            nc.sync.dma_start(out=bngbt[:, :].bitcast(F32R), in_=hbp(OFF_BG, [[2, 64], [1, 2]]))
            nc.sync.dma_start(out=nmt[:, :].bitcast(F32R), in_=hbp(OFF_NM, [[NJT, 128], [1, NJT]]))
            nc.sync.dma_start(out=hmt[:, :].bitcast(F32R), in_=hbp(OFF_HM, [[2, 64], [1, 2]]))
            nc.sync.dma_start(out=cst[:, :].bitcast(F32R), in_=hbp(OFF_CN, [[2, 1], [1, 2]]))
            nc.gpsimd.dma_start(out=gcam, in_=hbp(OFF_CN + 1, [[0, 64], [1, 1]]))

            nc.gpsimd.memset(feat[64:65, :].bitcast(F32), 1.0)
            nc.gpsimd.memset(kr4[:, :, :].bitcast(F32), 0.0)
            nc.gpsimd.memset(vT[:, :, 64:65].bitcast(F32), 1.0)
            for bf in (sabuf, scbuf):
                nc.gpsimd.memset(bf[0:64, :, 0:1].bitcast(F32), 0.0)
                nc.gpsimd.memset(bf[0:64, :, 65:66].bitcast(F32), 0.0)

            # ---- x half: zero pad cols, DMA real cols
            nc.gpsimd.memset(xh[:, :, :, :], 0.0)
            nc.sync.dma_start(
                out=xh[:, :, :, 1:65],
                in_=hap(OFF_XH, [[2 * XR * 64, 128], [XR * 64, 2],
                                 [64, XR], [1, 64]]))

            # ---- conv1 own half -> feat cols 64:2112 (raw y1) + stats
            stats1 = sm.tile([64, 4, 6], F32, tag="stats1")
            for T in range(4):
                pst = mcp.tile([64, 512], F32, tag="mc", name=f"c1ps{T}")
                for s in range(NTAPS):
                    tap, cb = divmod(s, 2)
                    dy, dx = divmod(tap, 3)
                    rhs = xh[:, cb, 8 * T + dy:8 * T + dy + 8, dx:dx + 64]
                    nc.tensor.matmul(pst, w1t[:, s, :], rhs,
                                     start=(s == 0), stop=(s == NTAPS - 1))
                nc.vector.bn_stats(stats1[:, T, :], pst)
                nc.vector.tensor_copy(feat[0:64, 64 + 512 * T:576 + 512 * T], pst)
            mv1 = sm.tile([64, 2], F32, tag="mv1")
            nc.vector.bn_aggr(mv1, stats1[:, :, :])

            # ---- pair-exchange raw y1, blend partner rows into feat
            y1d = dram.tile([64, MY], F32, tag="y1d")
            y1g = dram.tile([128, MY], F32, tag="y1g")
            nc.sync.dma_start(out=y1d[:, :], in_=feat[0:64, 64:2112].bitcast(F32))
            nc.gpsimd.collective_compute(
                "AllGather", ALU.bypass,
                replica_groups=[[0, 1], [2, 3], [4, 5], [6, 7]],
                ins=[y1d.opt()], outs=[y1g.opt()])
            # stage X: placement for h=0 receivers; stage Y: for h=1 receivers.
            # stage col s maps to feat col 2112+s (s<2112) or s-2112 (s>=2112).
            stX = sm.tile([64, WIN], F32, tag="stX")
            stY = sm.tile([64, WIN], F32, tag="stY")
            nc.vector.memset(stX[:, 2048:2176], 0.0)
            nc.vector.memset(stY[:, 0:128], 0.0)
            nc.sync.dma_start(out=stX[:, 0:2048], in_=y1g[64:128, :])
            nc.sync.dma_start(out=stY[:, 128:2112], in_=y1g[0:64, 0:1984])
            nc.sync.dma_start(out=stY[:, 2112:2176], in_=y1g[0:64, 1984:2048])
            nc.vector.tensor_scalar_mul(stX, stX, hmt[:, 1:2])   # keep iff h==0
            nc.vector.tensor_scalar_mul(stY, stY, hmt[:, 0:1])   # keep iff h==1
            nc.vector.tensor_tensor(feat[0:64, 2112:4224],
                                    stX[:, 0:2112], stY[:, 0:2112], ALU.add)
            nc.vector.tensor_tensor(feat[0:64, 0:64],
                                    stX[:, 2112:2176], stY[:, 2112:2176],
                                    ALU.add)

            def bn_coeffs(gl, tag):
                """gl [64,2] = (sum, sumsq) -> (scale, shift) [64,1] f32."""
                mean = sm.tile([64, 1], F32, tag=tag + "m", name=tag + "m")
                var = sm.tile([64, 1], F32, tag=tag + "v", name=tag + "v")
                scl = sm.tile([64, 1], F32, tag=tag + "s", name=tag + "s")
                sh = sm.tile([64, 1], F32, tag=tag + "h", name=tag + "h")
                nc.vector.tensor_scalar_mul(mean, gl[:, 0:1], 1.0 / N_STAT)
                nc.vector.tensor_scalar_mul(var, gl[:, 1:2], 1.0 / N_STAT)
                nc.vector.tensor_tensor(scl, mean, mean, ALU.mult)
                nc.vector.tensor_tensor(var, var, scl, ALU.subtract)
                nc.scalar.activation(var, var, AF.Sqrt, bias=epst, scale=1.0)
                nc.vector.reciprocal(var, var)
                nc.vector.tensor_tensor(scl, bngbt[:, 0:1], var, ALU.mult)
                nc.vector.tensor_tensor(sh, mean, scl, ALU.mult)
                nc.vector.tensor_tensor(sh, bngbt[:, 1:2], sh, ALU.subtract)
                return scl, sh

            def stat_ar(mv, tag):
                """partial (mean,var over MY) -> AllReduce -> (sum,sumsq)."""
                ars = sm.tile([64, 2], F32, tag=tag + "s", name=tag + "s")
                t_t = sm.tile([64, 1], F32, tag=tag + "t", name=tag + "t")
                nc.vector.tensor_scalar_mul(ars[:, 0:1], mv[:, 0:1], float(MY))
                nc.vector.tensor_tensor(t_t, mv[:, 0:1], mv[:, 0:1], ALU.mult)
                nc.vector.tensor_tensor(t_t, mv[:, 1:2], t_t, ALU.add)
                nc.vector.tensor_scalar_mul(ars[:, 1:2], t_t, float(MY))
                a_in = dram.tile([64, 2], F32, tag=tag + "_in", name=tag + "_in")
                a_out = dram.tile([64, 2], F32, tag=tag + "_out",
                                  name=tag + "_out")
                nc.sync.dma_start(out=a_in[:, :], in_=ars)
                nc.gpsimd.collective_compute(
                    "AllReduce", ALU.add,
                    replica_groups=[list(range(NCORES))],
                    ins=[a_in.opt()], outs=[a_out.opt()])
                gl = sm.tile([64, 2], F32, tag=tag + "g", name=tag + "g")
                nc.sync.dma_start(out=gl, in_=a_out[:, :])
                return gl

            # AR1: bn1 stats -> relu(bn(y1)) over all 66 local rows
            gl1 = stat_ar(mv1, "ar1")
            sc1, sh1 = bn_coeffs(gl1, "bn1")
            for c0 in range(0, NP, 1056):
                sl = feat[0:64, c0:c0 + 1056]
                nc.scalar.activation(sl, sl, AF.Relu, bias=sh1, scale=sc1)

            # ---- qkv
            qkvtiles = [(t * 512, 512) for t in range(8)] + [(4096, 128)]
            for ti, (c0, cw) in enumerate(qkvtiles):
                qps = mcp.tile([80, cw], F32, tag="mc", name="qps")
                nc.tensor.matmul(qps, wqkvt, feat[:, c0:c0 + cw],
                                 start=True, stop=True)
                nc.vector.tensor_copy(qkv[:, c0:c0 + cw], qps)
            # qr: q replicated at partition groups; row 32g+8 = ones
            # (pairs with the ebias row in kr4 -> energy gets +ebias[j])
            for g in range(4):
                nc.sync.dma_start(out=qr[32 * g:32 * g + 8, :],
                                  in_=qkv[64:72, 0:WIN])
            for g in range(4):
                nc.sync.dma_start(
                    out=qr[32 * g + 8:32 * g + 9, :],
                    in_=hbp(OFF_EB + NP, [[NP, 1], [1, WIN]]))
            # kr4: k repartitioned per j-group; row 8 of each 32-block holds
            # the exp masking bias for that j-tile
            kbounce = dram.tile([8, NP], F32R, tag="kbounce", name="kbounce")
            nc.sync.dma_start(out=kbounce[:, :], in_=qkv[72:80, :])
            for u in range(4):
                ksrc = bass.AP(tensor=kbounce.tensor,
                               offset=kbounce.offset + u * 128,
                               ap=[[NP, 8], [512, 8], [1, 128]])
                nc.sync.dma_start(out=kr4[32 * u:32 * u + 8, 0:8, :], in_=ksrc)
                nc.sync.dma_start(
                    out=kr4[32 * u + 8:32 * u + 9, 0:8, :],
                    in_=hbp(OFF_EB + u * 128, [[512, 8], [1, 128]]))
            nc.sync.dma_start(out=kr4[0:8, 8, :], in_=kbounce[:, 4096:4224])
            nc.sync.dma_start(out=kr4[8:9, 8, :],
                              in_=hbp(OFF_EB + 4096, [[NP, 1], [1, 128]]))

            # ---- vT transpose (+ones col), 4 per psum bank
            for j0 in range(0, 32, 4):
                tp = mcp.tile([128, 4, 64], F32R, tag="mc", name=f"vtp{j0}")
                for k in range(4):
                    jt = j0 + k
                    nc.tensor.transpose(
                        tp[:, k, :],
                        qkv[0:64, jt * 128:(jt + 1) * 128],
                        idt[0:64, 0:64])
                nc.vector.tensor_copy(vT[:, j0:j0 + 4, 0:64], tp)
            tpl = mcp.tile([128, 64], F32R, tag="mc", name="vtpl")
            nc.tensor.transpose(tpl, qkv[0:64, 32 * 128:33 * 128],
                                idt[0:64, 0:64])
            nc.vector.tensor_copy(vT[:, 32, 0:64], tpl)

            # ================= interleaved attention + CAM emission ========
            def pam_pair(jg0, chunk_cb=None):
                """Emit energy/exp/pam for j-groups jg0, jg0+1 (or lone 8)."""
                jgs = [jg0] if jg0 == 8 else [jg0, jg0 + 1]
                for ici, (i0, iw) in enumerate(ICM):
                    pt = ptp.tile([65, iw], F32, tag="pt", name="pt")
                    nmm = sum(4 if j < 8 else 1 for j in jgs)
                    k = 0
                    for jg in jgs:
                        nu2 = 2 if jg < 8 else 1
                        for p in range(2 if jg < 8 else 1):
                            et_ps = ps.tile([128, 2, 512], F32, tag="ps",
                                            name="et_ps")
                            for u2 in range(nu2):
                                u = 2 * p + u2
                                nc.tensor.matmul(
                                    et_ps[:, u2, 0:iw],
                                    kr4[32 * u:32 * u + 32, jg, :],
                                    qr[32 * u:32 * u + 32, i0:i0 + iw],
                                    start=True, stop=True,
                                    tile_position=(32 * u, 0))
                            eT = etp.tile([128, 2, 512], F32R, tag="et",
                                          bufs=2, name="eT")
                            if nu2 == 2:
                                nc.scalar.activation(eT[:, :, 0:iw],
                                                     et_ps[:, :, 0:iw],
                                                     AF.Exp, bias=0.0,
                                                     scale=1.0)
                            else:
                                nc.scalar.activation(eT[:, 0, 0:iw],
                                                     et_ps[:, 0, 0:iw],
                                                     AF.Exp, bias=0.0,
                                                     scale=1.0)
                            for u2 in range(nu2):
                                jt = 4 * jg + 2 * p + u2
                                nc.tensor.matmul(pt, vT[:, jt, :],
                                                 eT[:, u2, 0:iw],
                                                 start=(k == 0),
                                                 stop=(k == nmm - 1))
                                k += 1
                    if jg0 == 0:
                        nc.vector.tensor_copy(pacc[:, i0:i0 + iw], pt)
                    else:
                        nc.vector.tensor_tensor(pacc[:, i0:i0 + iw],
                                                pacc[:, i0:i0 + iw], pt,
                                                ALU.add)
                    if chunk_cb is not None:
                        chunk_cb(ici, i0, iw)

            pam_pair(0)
            # fT transposes (CAM input), masked
            for jt in range(NJT):
                tp = mcp.tile([128, 64], F32R, tag="mc", name=f"ftp{jt}")
                nc.tensor.transpose(tp, feat[0:64, jt * 128:(jt + 1) * 128],
                                    idt[0:64, 0:64])
                nc.vector.tensor_scalar_mul(fT[:, jt, :], tp, nmt[:, jt:jt + 1])

            pam_pair(2)
            # CAM: ce (chunked), softmax, cattnT
            ce_sb = sm.tile([64, 64], F32, tag="ce_sb")
            for ci_, (j0, nj) in enumerate([(0, 9), (9, 8), (17, 8), (25, 8)]):
                ce_ps = mcp.tile([64, 64], F32, tag="mc", name=f"ce{ci_}")
                for k in range(nj):
                    jt = j0 + k
                    nc.tensor.matmul(ce_ps, fT[:, jt, :], fT[:, jt, :],
                                     start=(k == 0), stop=(k == nj - 1))
                if ci_ == 0:
                    nc.vector.tensor_copy(ce_sb, ce_ps)
                else:
                    nc.vector.tensor_tensor(ce_sb, ce_sb, ce_ps, ALU.add)
            rmin = sm.tile([64, 1], F32, tag="rmin")
            nc.vector.tensor_reduce(rmin, ce_sb, mybir.AxisListType.X, ALU.min)
            cu = sm.tile([64, 64], F32, tag="cu")
            nc.scalar.activation(cu, ce_sb, AF.Exp, bias=rmin, scale=-1.0)
            rs = sm.tile([64, 1], F32, tag="rs")
            nc.vector.tensor_reduce(rs, cu, mybir.AxisListType.X, ALU.add)
            nc.vector.reciprocal(rs, rs)
            cattn = sm.tile([64, 64], F32R, tag="cattn")
            nc.vector.tensor_scalar_mul(cattn, cu, rs)
            ctp = mcp.tile([64, 64], F32R, tag="mc", name="ctp")
            nc.tensor.transpose(ctp, cattn, idt[0:64, 0:64])
            cattnT = sm.tile([64, 64], F32R, tag="cattnT")
            nc.vector.tensor_copy(cattnT, ctp)

            pam_pair(4)
            # CAM apply + scbuf
            for (i0, iw) in IC:
                cam_ps = mcp.tile([64, iw], F32, tag="mc", name="cam_ps")
                nc.tensor.matmul(cam_ps, cattnT, feat[0:64, i0:i0 + iw],
                                 start=True, stop=True)
                tmpc = etp.tile([64, iw], F32R, tag="camt", bufs=3,
                                name="tmpc")
                nc.vector.tensor_scalar_mul(tmpc, cam_ps, gcam)
                r0, nr = i0 // W, iw // W
                nc.vector.tensor_tensor(
                    scbuf[0:64, r0:r0 + nr, 1:65],
                    tmpc[:, :].rearrange("p (r c) -> p r c", c=W),
                    feat[0:64, i0:i0 + iw].rearrange("p (r c) -> p r c", c=W),
                    ALU.add)
            nc.vector.tensor_scalar_mul(scbuf[0:64, 0, 1:65],
                                        scbuf[0:64, 0, 1:65], hmt[:, 0:1])
            nc.vector.tensor_scalar_mul(scbuf[0:64, 33, 1:65],
                                        scbuf[0:64, 33, 1:65], hmt[:, 1:2])
            for (a, b) in [(0, 9), (9, 17), (17, 25), (25, 33)]:
                nc.gpsimd.tensor_copy(scbuf[64:128, a:b, :],
                                      scbuf[0:64, a + 1:b + 1, :])

            def conv2(buf, y2sb, sttag):
                st = sm.tile([64, 4, 6], F32, tag=sttag, name=sttag)
                for T in range(4):
                    r0 = 1 + 8 * T
                    yps = mcp.tile([64, 512], F32, tag="mc", name="yps")
                    for dxi in range(3):
                        rhs1 = buf[:, r0 - 1:r0 + 7, dxi:dxi + 64]
                        nc.tensor.matmul(yps, w2at[:, dxi * 64:(dxi + 1) * 64],
                                         rhs1, start=(dxi == 0), stop=False)
                        rhs2 = buf[0:64, r0 + 1:r0 + 9, dxi:dxi + 64]
                        nc.tensor.matmul(yps, w2bt[:, dxi * 64:(dxi + 1) * 64],
                                         rhs2, start=False, stop=(dxi == 2))
                    nc.vector.bn_stats(st[:, T, :], yps)
                    nc.vector.tensor_copy(y2sb[:, T * 512:(T + 1) * 512], yps)
                mv = sm.tile([64, 2], F32, tag=sttag + "mv", name=sttag + "mv")
                nc.vector.bn_aggr(mv, st[:, :, :])
                return mv

            pam_pair(6)
            # conv2 on CAM branch + its stats AR (hidden under attention)
            mvb = conv2(scbuf, y2b, "stb")
            glb = stat_ar(mvb, "arb")
            scb, shb = bn_coeffs(glb, "bnb")
            rb = big.tile([64, MY], F32R, tag="rb")
            nc.scalar.activation(rb, y2b, AF.Relu, bias=shb, scale=scb)

            # ---- pam normalize (r = gamma_pam / s), sa = pam_u*r + feat1
            def pam_div(src, i0, iw, sfx):
                r32 = sm.tile([1, iw], F32, tag="r32", name="r32" + sfx)
                nc.vector.reciprocal(r32, src[64:65, :])
                rr = sm.tile([1, iw], F32R, tag="rr", name="rr" + sfx)
                nc.vector.tensor_scalar_mul(rr, r32, cst[0:1, 0:1])
                rbc = etp.tile([64, iw], F32R, tag="camt", bufs=3,
                               name="rbc" + sfx)
                nc.gpsimd.partition_broadcast(rbc, rr)
                tmpa = etp.tile([64, iw], F32R, tag="camt", bufs=3,
                                name="tmpa" + sfx)
                nc.vector.tensor_tensor(tmpa, src[0:64, :], rbc, ALU.mult)
                r0, nr = i0 // W, iw // W
                nc.vector.tensor_tensor(
                    sabuf[0:64, r0:r0 + nr, 1:65],
                    tmpa[:, :].rearrange("p (r c) -> p r c", c=W),
                    feat[0:64, i0:i0 + iw].rearrange("p (r c) -> p r c", c=W),
                    ALU.add)

            pam_pair(8, chunk_cb=lambda ici, i0, iw: pam_div(
                pacc[:, i0:i0 + iw], i0, iw, str(ici)))
            nc.vector.tensor_scalar_mul(sabuf[0:64, 0, 1:65],
                                        sabuf[0:64, 0, 1:65], hmt[:, 0:1])
            nc.vector.tensor_scalar_mul(sabuf[0:64, 33, 1:65],
                                        sabuf[0:64, 33, 1:65], hmt[:, 1:2])
            for (a, b) in [(0, 9), (9, 17), (17, 25), (25, 33)]:
                nc.gpsimd.tensor_copy(sabuf[64:128, a:b, :],
                                      sabuf[0:64, a + 1:b + 1, :])

            mva = conv2(sabuf, y2a, "sta")
            gla = stat_ar(mva, "ara")
            sca, sha = bn_coeffs(gla, "bna")

            # ---- relu + sum -> fp16 feat_sum; conv8 (1x1, 64->256) runs on
            # the host after download: 4x fewer D2H bytes over the tunnel.
            for T in range(4):
                sl = slice(T * 512, (T + 1) * 512)
                ra = etp.tile([64, 512], F32R, tag="camt", bufs=3,
                              name=f"ra{T}")
                nc.scalar.activation(ra, y2a[:, sl], AF.Relu,
                                     bias=sha, scale=sca)
                nc.vector.tensor_tensor(fs16[:, sl], ra, rb[:, sl], ALU.add)
                nc.sync.dma_start(out=out[:, sl], in_=fs16[:, sl])
    nc.finalize()
    return nc


# ---------------------------------------------------------------- runner
class _Runner:
    def __init__(self, nc, n_cores=NCORES):
        install_neuronx_cc_hook()
        self.nc = nc
        self.n_cores = n_cores
        in_names, out_names, out_avals, zero_shapes = [], [], [], []
        pname = nc.partition_id_tensor.name if nc.partition_id_tensor else None
        for alloc in nc.m.functions[0].allocations:
            if not isinstance(alloc, mybir.MemoryLocationSet):
                continue
            name = alloc.memorylocations[0].name
            if alloc.kind == "ExternalInput":
                if name != pname:
                    in_names.append(name)
            elif alloc.kind == "ExternalOutput":
                out_names.append(name)
                shape = tuple(alloc.tensor_shape)
                dtype = mybir.dt.np(alloc.dtype)
                out_avals.append(jax.core.ShapedArray(shape, dtype))
                zero_shapes.append((shape, dtype))
        self.n_params = len(in_names)
        self.in_names = in_names + out_names
        if pname is not None:
            self.in_names.append(pname)
        self.out_names = out_names

        devices = jax.devices()[:n_cores]
        self.mesh = Mesh(np.asarray(devices), ("core",))
        self.sharding = NamedSharding(self.mesh, PartitionSpec("core"))

        in_names_t = tuple(self.in_names)
        out_names_t = tuple(out_names)
        out_avals_t = tuple(out_avals)
        has_pid = pname is not None

        def _body(*args):
            operands = list(args)
            if has_pid:
                operands.append(partition_id_tensor())
            outs = _bass_exec_p.bind(
                *operands,
                out_avals=out_avals_t,
                in_names=in_names_t,
                out_names=out_names_t,
                lowering_input_output_aliases=(),
                sim_require_finite=True,
                sim_require_nnan=True,
                nc=nc,
            )
            return tuple(outs)

        n_args = self.n_params + len(out_names)
        self.fn = jax.jit(
            shard_map(_body, mesh=self.mesh,
                      in_specs=(PartitionSpec("core"),) * n_args,
                      out_specs=(PartitionSpec("core"),) * len(out_names),
                      check_rep=False),
            keep_unused=True,
        )
        # cached placeholder "output" operands: device-resident, never
        # donated, never transferred again. The kernel writes every output
        # element so their contents are irrelevant.
        self.placeholders = [
            jax.jit(lambda s=shape, d=dtype: jnp.zeros((n_cores * s[0],
                                                        *s[1:]), d),
                    out_shardings=self.sharding)()
            for shape, dtype in zero_shapes
        ]

    def __call__(self, *concat_inputs):
        dev_inputs = [jax.device_put(a, self.sharding) for a in concat_inputs]
        outs = self.fn(*dev_inputs, *self.placeholders)
        return [np.asarray(o) for o in outs]


_CACHE = {}


def kernel(**inputs):
    if "runner" not in _CACHE:
        _CACHE["runner"] = _Runner(_build())
    runner = _CACHE["runner"]
    ha, hb = _prep_core_inputs(
        np.asarray(inputs["x"], np.float32), np.asarray(inputs["w1"]),
        np.asarray(inputs["bn_g"]), np.asarray(inputs["bn_b"]),
        np.asarray(inputs["wq"]), np.asarray(inputs["bq"]),
        np.asarray(inputs["wk"]), np.asarray(inputs["bk"]),
        np.asarray(inputs["wv"]), np.asarray(inputs["bv"]),
        np.asarray(inputs["gamma_pam"]), np.asarray(inputs["gamma_cam"]),
        np.asarray(inputs["w2"]), np.asarray(inputs["w8"]),
        np.asarray(inputs["b8"]))
    res = runner(ha, hb)
    fs = res[0].reshape(NCORES, 64, MY).astype(np.float32)
    w8f = np.asarray(inputs["w8"], np.float32)[:, :, 0, 0]       # [256, 64]
    b8f = np.asarray(inputs["b8"], np.float32)[:, None]          # [256, 1]
    # conv8 = 1x1 conv: one GEMM over all cores' columns
    og = (w8f @ fs.transpose(1, 0, 2).reshape(64, NCORES * MY)) + b8f
    og = og.reshape(CO, NCORES, 32, W)
    out = np.empty((B, CO, H, W), np.float32)
    for c in range(NCORES):
        b, h = divmod(c, 2)
        out[b, :, 32 * h:32 * h + 32, :] = og[:, c]
    return out


# revision 11
# speedup vs baseline: 1.6234x; 1.1733x over previous
"""DANetHead Trainium2 kernel: 8-core SPMD (batch x row-half sharding).

Self-contained: hardcodes all shapes from the problem spec.

Per-core layout (core c: sample b=c//2, half h=c%2):
  P = [-1, 0..63, 64] (66 padded rows; -1/64 zero).
  feat local row L (0..65) holds padded row P[(L+32h) % 66] (cyclic rotation,
  so every core's attention/conv2 window is local rows 0..33 uniformly).
  window = local rows 0..33 (flat 0..2175); my output rows = 1..32.

Transfer-optimized: each core uploads only its own half of x (+1 halo row)
in fp16 inside a single packed blob; conv1 runs on the half, then the raw
conv1 outputs are pair-AllGathered on device and blended (per-core scalar
masks select the h=0/h=1 placement) into the full rotated feat layout.
Output is fp16. A custom PJRT runner avoids uploading donated zero output
buffers (the kernel writes every output element).
"""
import numpy as np

import jax
import jax.numpy as jnp
from jax.sharding import Mesh, PartitionSpec, NamedSharding
from jax.experimental.shard_map import shard_map

import concourse.bass as bass
import concourse.tile as tile
from concourse import bacc, mybir
from concourse.bass2jax import (_bass_exec_p, install_neuronx_cc_hook,
                                partition_id_tensor)

F32 = mybir.dt.float32
F32R = mybir.dt.float32r
F16 = mybir.dt.float16
AF = mybir.ActivationFunctionType
ALU = mybir.AluOpType

B, CIN, H, W = 4, 256, 64, 64
CI, CQ, CO = 64, 8, 256
NCORES = 8
LR = 66                  # local feat rows
NP = LR * W              # 4224
NJT = NP // 128          # 33 j-tiles
WIN = 34 * W             # 2176
MY = 32 * W              # 2048
NTAPS = 18               # 9 taps x 2 cin blocks
XR = 34                  # x rows per core (own 32 + halo)
# i chunks: CAM uses full window; PAM main loop uses ICM + tail
IC = [(0, 512), (512, 512), (1024, 512), (1536, 512), (2048, 128)]
ICM = [(0, 512), (512, 512), (1024, 512), (1536, 384), (1920, 256)]
N_STAT = 16384.0

# blob A (fp16) element offsets
OFF_XH = 0
OFF_W1 = OFF_XH + 128 * 2 * XR * 64      # 557056
OFF_W2A = OFF_W1 + 128 * NTAPS * CI      # 704512
OFF_W2B = OFF_W2A + 128 * 3 * CI         # 729088
OFF_QKV = OFF_W2B + 64 * 3 * CI          # 741376
OFF_ID = OFF_QKV + 65 * 80               # 746576
WT_END = OFF_ID + 128 * 128              # 762960
WT_LEN = WT_END - OFF_W1                 # 205904 shared fp16 elements
WT_CH = WT_LEN // NCORES                 # 25738 per-core chunk
# relative offsets inside the gathered weight tail
R_W1 = 0
R_W2A = 128 * NTAPS * CI                 # 147456
R_W2B = R_W2A + 128 * 3 * CI             # 172032
R_QKV = R_W2B + 64 * 3 * CI              # 184320
R_ID = R_QKV + 65 * 80                   # 189520
KA = OFF_W1 + WT_CH                      # 582794

# blob B (f32) element offsets
OFF_EB = 0
OFF_NM = OFF_EB + 2 * NP                 # 8448
OFF_HM = OFF_NM + 128 * NJT              # 12672
OFF_BG = OFF_HM + 64 * 2                 # 12800
OFF_CN = OFF_BG + 64 * 2                 # 12928
KB = OFF_CN + 2                          # 12930

PAD = [-1] + list(range(64)) + [64]


# ---------------------------------------------------------------- host prep
def _shared_blobs():
    """(blobA weight tail [KA-OFF_W1] fp16, blobB per-h variants [2, KB] f32)."""
    return None


def _prep_core_inputs(x, w1, bn_g, bn_b, wq, bq, wk, bk, wv, bv,
                      gamma_pam, gamma_cam, w2, w8, b8):
    f = np.float32
    # ---- shared weight tail of blob A (fp16)
    w1s = np.zeros((128, NTAPS, CI), f)
    for dy in range(3):
        for dx in range(3):
            for cb in range(2):
                s = (dy * 3 + dx) * 2 + cb
                w1s[:, s, :] = w1[:, cb * 128:(cb + 1) * 128, dy, dx].T
    w2a = np.zeros((128, 3, CI), f)
    w2b = np.zeros((64, 3, CI), f)
    for dx in range(3):
        w2a[:64, dx, :] = w2[:, :, 0, dx].T
        w2a[64:, dx, :] = w2[:, :, 1, dx].T
        w2b[:, dx, :] = w2[:, :, 2, dx].T
    wqkv = np.zeros((65, 80), f)
    wqkv[:64, 0:64] = wv[:, :, 0, 0].T
    wqkv[:64, 64:72] = wq[:, :, 0, 0].T
    wqkv[:64, 72:80] = wk[:, :, 0, 0].T
    wqkv[64, 0:64] = bv
    wqkv[64, 64:72] = bq
    wqkv[64, 72:80] = bk
    wtail = np.concatenate([
        w1s.ravel(), w2a.ravel(), w2b.ravel(), wqkv.ravel(),
        np.eye(128, dtype=f).ravel()]).astype(np.float16)

    # ---- blob B per-h variants (f32)
    bngb = np.stack([bn_g, bn_b], 1).astype(f)
    consts = np.array([float(gamma_pam[0]), float(gamma_cam[0])], f)
    hbv = np.zeros((2, KB), f)
    for h in range(2):
        centers = [PAD[(L + 32 * h) % 66] for L in range(LR)]
        realp = np.repeat(np.array([0 <= g <= 63 for g in centers]), W)
        ebias = np.concatenate([np.where(realp, 0.0, -1000.0).astype(f),
                                np.ones(NP, f)])
        nmask = np.where(realp, 1.0, 0.0).astype(f).reshape(NJT, 128).T
        hmask = np.zeros((64, 2), f)
        hmask[:, 0] = 0.0 if h == 0 else 1.0
        hmask[:, 1] = 0.0 if h == 1 else 1.0
        hbv[h] = np.concatenate([ebias, nmask.ravel(), hmask.ravel(),
                                 bngb.ravel(), consts])

    # ---- per-core blob A: xh [128, 2, 34, 64] fp16 + shared weight tail
    x16 = np.asarray(x, np.float16)
    ha = np.empty((NCORES, KA), np.float16)
    ha[:, OFF_W1:] = wtail.reshape(NCORES, WT_CH)
    hb = np.empty((NCORES, KB), f)
    for c in range(NCORES):
        b, h = divmod(c, 2)
        xh = np.zeros((128, 2, XR, 64), np.float16)
        if h == 0:
            # rows r=1..33 <- image rows 0..32 (r=0 is the zero pad row)
            xh[:, 0, 1:34, :] = x16[b, :128, 0:33, :]
            xh[:, 1, 1:34, :] = x16[b, 128:, 0:33, :]
        else:
            # rows r=0..32 <- image rows 31..63 (r=33 is the zero pad row)
            xh[:, 0, 0:33, :] = x16[b, :128, 31:64, :]
            xh[:, 1, 0:33, :] = x16[b, 128:, 31:64, :]
        ha[c, :OFF_W1] = xh.ravel()
        hb[c] = hbv[h]
    return ha, hb


# ---------------------------------------------------------------- bass build
def _build():
    nc = bacc.Bacc()
    ha = nc.declare_dram_parameter("ha", [1, KA], F16, isOutput=False)
    hb = nc.declare_dram_parameter("hb", [1, KB], F32R, isOutput=False)
    out = nc.declare_dram_parameter("out", [64, MY], F16, isOutput=True)

    def hap(off, ap):
        return bass.AP(tensor=ha, offset=off, ap=ap)

    def hbp(off, ap):
        return bass.AP(tensor=hb, offset=off, ap=ap)

    with tile.TileContext(nc) as tc:
        with tc.tile_pool(name="big", bufs=1) as big, \
             tc.tile_pool(name="wt", bufs=1) as wt, \
             tc.tile_pool(name="sm", bufs=2) as sm, \
             tc.tile_pool(name="et", bufs=2) as etp, \
             tc.tile_pool(name="ps", bufs=2, space="PSUM") as ps, \
             tc.tile_pool(name="pt", bufs=2, space="PSUM") as ptp, \
             tc.tile_pool(name="mc", bufs=2, space="PSUM") as mcp, \
             tc.tile_pool(name="dram", bufs=1, space="DRAM") as dram:

            # ---- persistent sbuf tensors
            feat = big.tile([65, NP], F32R, tag="feat")   # y1 then feat1(+ones)
            qkv = big.tile([80, NP], F32R, tag="qkv")
            qr = big.tile([128, WIN], F32R, tag="qr")
            kr4 = big.tile([128, 9, 128], F32R, tag="kr4")
            vT = big.tile([128, NJT, 65], F32R, tag="vT")
            fT = big.tile([128, NJT, CI], F32R, tag="fT")
            sabuf = big.tile([128, 34, LR], F32R, tag="sabuf")
            scbuf = big.tile([128, 34, LR], F32R, tag="scbuf")
            y2a = big.tile([64, MY], F32, tag="y2a")
            y2b = big.tile([64, MY], F32, tag="y2b")
            fs16 = big.tile([64, MY], F16, tag="fs16")
            pacc = big.tile([65, WIN], F32, tag="pacc")   # pam accumulator
            xh = big.tile([128, 2, XR, LR], F16, tag="xh")

            # ---- weights / consts in sbuf
            w1t = wt.tile([128, NTAPS, CI], F16, tag="w1t")
            wqkvt = wt.tile([65, 80], F32R, tag="wqkvt")
            w2at = wt.tile([128, 3 * CI], F32R, tag="w2at")
            w2bt = wt.tile([64, 3 * CI], F32R, tag="w2bt")
            bngbt = wt.tile([64, 2], F32, tag="bngbt")
            nmt = wt.tile([128, NJT], F32, tag="nmt")
            hmt = wt.tile([64, 2], F32, tag="hmt")
            cst = wt.tile([1, 2], F32, tag="cst")
            gcam = wt.tile([64, 1], F32, tag="gcam")
            epst = wt.tile([64, 1], F32, tag="epst")
            nc.vector.memset(epst, 1e-5)
            idt = wt.tile([128, 128], F32R, tag="idt")
            # fp16 staging for converted weights
            wq16 = wt.tile([65, 80], F16, tag="wq16")
            w2a16 = wt.tile([128, 3 * CI], F16, tag="w2a16")
            w2b16 = wt.tile([64, 3 * CI], F16, tag="w2b16")
            id16 = wt.tile([128, 128], F16, tag="id16")

            # reassemble the shared weight tail on device: each core ships
            # 1/8th, pair of DMAs bounce it to DRAM, all-8 AllGather restores
            # the full tail on every core.
            wcin = dram.tile([2, WT_CH // 2], F16, tag="wcin")
            nc.sync.dma_start(out=wcin[:, :], in_=hap(OFF_W1, [[WT_CH // 2, 2],
                                                               [1, WT_CH // 2]]))
            wg = dram.tile([2 * NCORES, WT_CH // 2], F16, tag="wg")
            nc.gpsimd.collective_compute(
                "AllGather", ALU.bypass,
                replica_groups=[list(range(NCORES))],
                ins=[wcin.opt()], outs=[wg.opt()])

            def wgp(off, ap):
                return bass.AP(tensor=wg.tensor, offset=wg.offset + off, ap=ap)

            nc.sync.dma_start(out=w1t, in_=wgp(R_W1, [[NTAPS * CI, 128],
                                                      [CI, NTAPS], [1, CI]]))
            nc.sync.dma_start(out=w2a16, in_=wgp(R_W2A, [[192, 128], [1, 192]]))
            nc.sync.dma_start(out=w2b16, in_=wgp(R_W2B, [[192, 64], [1, 192]]))
            nc.sync.dma_start(out=wq16, in_=wgp(R_QKV, [[80, 65], [1, 80]]))
            nc.sync.dma_start(out=id16, in_=wgp(R_ID, [[128, 128], [1, 128]]))
            nc.vector.tensor_copy(w2at, w2a16)
            nc.vector.tensor_copy(w2bt, w2b16)
            nc.vector.tensor_copy(wqkvt, wq16)
            nc.vector.tensor_copy(idt, id16)

            nc.sync.dma_start(out=bngbt[:, :].bitcast(F32R), in_=hbp(OFF_BG, [[2, 64], [1, 2]]))
            nc.sync.dma_start(out=nmt[:, :].bitcast(F32R), in_=hbp(OFF_NM, [[NJT, 128], [1, NJT]]))
            nc.sync.dma_start(out=hmt[:, :].bitcast(F32R), in_=hbp(OFF_HM, [[2, 64], [1, 2]]))
            nc.sync.dma_start(out=cst[:, :].bitcast(F32R), in_=hbp(OFF_CN, [[2, 1], [1, 2]]))
            nc.gpsimd.dma_start(out=gcam, in_=hbp(OFF_CN + 1, [[0, 64], [1, 1]]))

            nc.gpsimd.memset(feat[64:65, :].bitcast(F32), 1.0)
            nc.gpsimd.memset(kr4[:, :, :].bitcast(F32), 0.0)
            nc.gpsimd.memset(vT[:, :, 64:65].bitcast(F32), 1.0)
            for bf in (sabuf, scbuf):
                nc.gpsimd.memset(bf[0:64, :, 0:1].bitcast(F32), 0.0)
                nc.gpsimd.memset(bf[0:64, :, 65:66].bitcast(F32), 0.0)

            # ---- x half: zero pad cols, DMA real cols
            nc.gpsimd.memset(xh[:, :, :, :], 0.0)
            nc.sync.dma_start(
                out=xh[:, :, :, 1:65],
                in_=hap(OFF_XH, [[2 * XR * 64, 128], [XR * 64, 2],
                                 [64, XR], [1, 64]]))

            # ---- conv1 own half -> feat cols 64:2112 (raw y1) + stats
            stats1 = sm.tile([64, 4, 6], F32, tag="stats1")
            for T in range(4):
                pst = mcp.tile([64, 512], F32, tag="mc", name=f"c1ps{T}")
                for s in range(NTAPS):
                    tap, cb = divmod(s, 2)
                    dy, dx = divmod(tap, 3)
                    rhs = xh[:, cb, 8 * T + dy:8 * T + dy + 8, dx:dx + 64]
                    nc.tensor.matmul(pst, w1t[:, s, :], rhs,
                                     start=(s == 0), stop=(s == NTAPS - 1))
                nc.vector.bn_stats(stats1[:, T, :], pst)
                nc.vector.tensor_copy(feat[0:64, 64 + 512 * T:576 + 512 * T], pst)
            mv1 = sm.tile([64, 2], F32, tag="mv1")
            nc.vector.bn_aggr(mv1, stats1[:, :, :])

            # ---- pair-exchange raw y1, blend partner rows into feat
            y1d = dram.tile([64, MY], F32, tag="y1d")
            y1g = dram.tile([128, MY], F32, tag="y1g")
            nc.sync.dma_start(out=y1d[:, :], in_=feat[0:64, 64:2112].bitcast(F32))
            nc.gpsimd.collective_compute(
                "AllGather", ALU.bypass,
                replica_groups=[[0, 1], [2, 3], [4, 5], [6, 7]],
                ins=[y1d.opt()], outs=[y1g.opt()])
            # stage X: placement for h=0 receivers; stage Y: for h=1 receivers.
            # stage col s maps to feat col 2112+s (s<2112) or s-2112 (s>=2112).
            stX = sm.tile([64, WIN], F32, tag="stX")
            stY = sm.tile([64, WIN], F32, tag="stY")
            nc.vector.memset(stX[:, 2048:2176], 0.0)
            nc.vector.memset(stY[:, 0:128], 0.0)
            nc.sync.dma_start(out=stX[:, 0:2048], in_=y1g[64:128, :])
            nc.sync.dma_start(out=stY[:, 128:2112], in_=y1g[0:64, 0:1984])
            nc.sync.dma_start(out=stY[:, 2112:2176], in_=y1g[0:64, 1984:2048])
            nc.vector.tensor_scalar_mul(stX, stX, hmt[:, 1:2])   # keep iff h==0
            nc.vector.tensor_scalar_mul(stY, stY, hmt[:, 0:1])   # keep iff h==1
            nc.vector.tensor_tensor(feat[0:64, 2112:4224],
                                    stX[:, 0:2112], stY[:, 0:2112], ALU.add)
            nc.vector.tensor_tensor(feat[0:64, 0:64],
                                    stX[:, 2112:2176], stY[:, 2112:2176],
                                    ALU.add)

            def bn_coeffs(gl, tag):
                """gl [64,2] = (sum, sumsq) -> (scale, shift) [64,1] f32."""
                mean = sm.tile([64, 1], F32, tag=tag + "m", name=tag + "m")
                var = sm.tile([64, 1], F32, tag=tag + "v", name=tag + "v")
                scl = sm.tile([64, 1], F32, tag=tag + "s", name=tag + "s")
                sh = sm.tile([64, 1], F32, tag=tag + "h", name=tag + "h")
                nc.vector.tensor_scalar_mul(mean, gl[:, 0:1], 1.0 / N_STAT)
                nc.vector.tensor_scalar_mul(var, gl[:, 1:2], 1.0 / N_STAT)
                nc.vector.tensor_tensor(scl, mean, mean, ALU.mult)
                nc.vector.tensor_tensor(var, var, scl, ALU.subtract)
                nc.scalar.activation(var, var, AF.Sqrt, bias=epst, scale=1.0)
                nc.vector.reciprocal(var, var)
                nc.vector.tensor_tensor(scl, bngbt[:, 0:1], var, ALU.mult)
                nc.vector.tensor_tensor(sh, mean, scl, ALU.mult)
                nc.vector.tensor_tensor(sh, bngbt[:, 1:2], sh, ALU.subtract)
                return scl, sh

            def stat_ar(mv, tag):
                """partial (mean,var over MY) -> AllReduce -> (sum,sumsq)."""
                ars = sm.tile([64, 2], F32, tag=tag + "s", name=tag + "s")
                t_t = sm.tile([64, 1], F32, tag=tag + "t", name=tag + "t")
                nc.vector.tensor_scalar_mul(ars[:, 0:1], mv[:, 0:1], float(MY))
                nc.vector.tensor_tensor(t_t, mv[:, 0:1], mv[:, 0:1], ALU.mult)
                nc.vector.tensor_tensor(t_t, mv[:, 1:2], t_t, ALU.add)
                nc.vector.tensor_scalar_mul(ars[:, 1:2], t_t, float(MY))
                a_in = dram.tile([64, 2], F32, tag=tag + "_in", name=tag + "_in")
                a_out = dram.tile([64, 2], F32, tag=tag + "_out",
                                  name=tag + "_out")
                nc.sync.dma_start(out=a_in[:, :], in_=ars)
                nc.gpsimd.collective_compute(
                    "AllReduce", ALU.add,
                    replica_groups=[list(range(NCORES))],
                    ins=[a_in.opt()], outs=[a_out.opt()])
                gl = sm.tile([64, 2], F32, tag=tag + "g", name=tag + "g")
                nc.sync.dma_start(out=gl, in_=a_out[:, :])
                return gl

            # AR1: bn1 stats -> relu(bn(y1)) over all 66 local rows
            gl1 = stat_ar(mv1, "ar1")
            sc1, sh1 = bn_coeffs(gl1, "bn1")
            for c0 in range(0, NP, 1056):
                sl = feat[0:64, c0:c0 + 1056]
                nc.scalar.activation(sl, sl, AF.Relu, bias=sh1, scale=sc1)

            # ---- qkv
            qkvtiles = [(t * 512, 512) for t in range(8)] + [(4096, 128)]
            for ti, (c0, cw) in enumerate(qkvtiles):
                qps = mcp.tile([80, cw], F32, tag="mc", name="qps")
                nc.tensor.matmul(qps, wqkvt, feat[:, c0:c0 + cw],
                                 start=True, stop=True)
                nc.vector.tensor_copy(qkv[:, c0:c0 + cw], qps)
            # qr: q replicated at partition groups; row 32g+8 = ones
            # (pairs with the ebias row in kr4 -> energy gets +ebias[j])
            for g in range(4):
                nc.sync.dma_start(out=qr[32 * g:32 * g + 8, :],
                                  in_=qkv[64:72, 0:WIN])
            for g in range(4):
                nc.sync.dma_start(
                    out=qr[32 * g + 8:32 * g + 9, :],
                    in_=hbp(OFF_EB + NP, [[NP, 1], [1, WIN]]))
            # kr4: k repartitioned per j-group; row 8 of each 32-block holds
            # the exp masking bias for that j-tile
            kbounce = dram.tile([8, NP], F32R, tag="kbounce", name="kbounce")
            nc.sync.dma_start(out=kbounce[:, :], in_=qkv[72:80, :])
            for u in range(4):
                ksrc = bass.AP(tensor=kbounce.tensor,
                               offset=kbounce.offset + u * 128,
                               ap=[[NP, 8], [512, 8], [1, 128]])
                nc.sync.dma_start(out=kr4[32 * u:32 * u + 8, 0:8, :], in_=ksrc)
                nc.sync.dma_start(
                    out=kr4[32 * u + 8:32 * u + 9, 0:8, :],
                    in_=hbp(OFF_EB + u * 128, [[512, 8], [1, 128]]))
            nc.sync.dma_start(out=kr4[0:8, 8, :], in_=kbounce[:, 4096:4224])
            nc.sync.dma_start(out=kr4[8:9, 8, :],
                              in_=hbp(OFF_EB + 4096, [[NP, 1], [1, 128]]))

            # ---- vT transpose (+ones col), 4 per psum bank
            for j0 in range(0, 32, 4):
                tp = mcp.tile([128, 4, 64], F32R, tag="mc", name=f"vtp{j0}")
                for k in range(4):
                    jt = j0 + k
                    nc.tensor.transpose(
                        tp[:, k, :],
                        qkv[0:64, jt * 128:(jt + 1) * 128],
                        idt[0:64, 0:64])
                nc.vector.tensor_copy(vT[:, j0:j0 + 4, 0:64], tp)
            tpl = mcp.tile([128, 64], F32R, tag="mc", name="vtpl")
            nc.tensor.transpose(tpl, qkv[0:64, 32 * 128:33 * 128],
                                idt[0:64, 0:64])
            nc.vector.tensor_copy(vT[:, 32, 0:64], tpl)

            # ================= interleaved attention + CAM emission ========
            def pam_pair(jg0, chunk_cb=None):
                """Emit energy/exp/pam for j-groups jg0, jg0+1 (or lone 8)."""
                jgs = [jg0] if jg0 == 8 else [jg0, jg0 + 1]
                for ici, (i0, iw) in enumerate(ICM):
                    pt = ptp.tile([65, iw], F32, tag="pt", name="pt")
                    nmm = sum(4 if j < 8 else 1 for j in jgs)
                    k = 0
                    for jg in jgs:
                        nu2 = 2 if jg < 8 else 1
                        for p in range(2 if jg < 8 else 1):
                            et_ps = ps.tile([128, 2, 512], F32, tag="ps",
                                            name="et_ps")
                            for u2 in range(nu2):
                                u = 2 * p + u2
                                nc.tensor.matmul(
                                    et_ps[:, u2, 0:iw],
                                    kr4[32 * u:32 * u + 32, jg, :],
                                    qr[32 * u:32 * u + 32, i0:i0 + iw],
                                    start=True, stop=True,
                                    tile_position=(32 * u, 0))
                            eT = etp.tile([128, 2, 512], F32R, tag="et",
                                          bufs=2, name="eT")
                            if nu2 == 2:
                                nc.scalar.activation(eT[:, :, 0:iw],
                                                     et_ps[:, :, 0:iw],
                                                     AF.Exp, bias=0.0,
                                                     scale=1.0)
                            else:
                                nc.scalar.activation(eT[:, 0, 0:iw],
                                                     et_ps[:, 0, 0:iw],
                                                     AF.Exp, bias=0.0,
                                                     scale=1.0)
                            for u2 in range(nu2):
                                jt = 4 * jg + 2 * p + u2
                                nc.tensor.matmul(pt, vT[:, jt, :],
                                                 eT[:, u2, 0:iw],
                                                 start=(k == 0),
                                                 stop=(k == nmm - 1))
                                k += 1
                    if jg0 == 0:
                        nc.vector.tensor_copy(pacc[:, i0:i0 + iw], pt)
                    else:
                        nc.vector.tensor_tensor(pacc[:, i0:i0 + iw],
                                                pacc[:, i0:i0 + iw], pt,
                                                ALU.add)
                    if chunk_cb is not None:
                        chunk_cb(ici, i0, iw)

            pam_pair(0)
            # fT transposes (CAM input), masked
            for jt in range(NJT):
                tp = mcp.tile([128, 64], F32R, tag="mc", name=f"ftp{jt}")
                nc.tensor.transpose(tp, feat[0:64, jt * 128:(jt + 1) * 128],
                                    idt[0:64, 0:64])
                nc.vector.tensor_scalar_mul(fT[:, jt, :], tp, nmt[:, jt:jt + 1])

            pam_pair(2)
            # CAM: ce (chunked), softmax, cattnT
            ce_sb = sm.tile([64, 64], F32, tag="ce_sb")
            for ci_, (j0, nj) in enumerate([(0, 9), (9, 8), (17, 8), (25, 8)]):
                ce_ps = mcp.tile([64, 64], F32, tag="mc", name=f"ce{ci_}")
                for k in range(nj):
                    jt = j0 + k
                    nc.tensor.matmul(ce_ps, fT[:, jt, :], fT[:, jt, :],
                                     start=(k == 0), stop=(k == nj - 1))
                if ci_ == 0:
                    nc.vector.tensor_copy(ce_sb, ce_ps)
                else:
                    nc.vector.tensor_tensor(ce_sb, ce_sb, ce_ps, ALU.add)
            rmin = sm.tile([64, 1], F32, tag="rmin")
            nc.vector.tensor_reduce(rmin, ce_sb, mybir.AxisListType.X, ALU.min)
            cu = sm.tile([64, 64], F32, tag="cu")
            nc.scalar.activation(cu, ce_sb, AF.Exp, bias=rmin, scale=-1.0)
            rs = sm.tile([64, 1], F32, tag="rs")
            nc.vector.tensor_reduce(rs, cu, mybir.AxisListType.X, ALU.add)
            nc.vector.reciprocal(rs, rs)
            cattn = sm.tile([64, 64], F32R, tag="cattn")
            nc.vector.tensor_scalar_mul(cattn, cu, rs)
            ctp = mcp.tile([64, 64], F32R, tag="mc", name="ctp")
            nc.tensor.transpose(ctp, cattn, idt[0:64, 0:64])
            cattnT = sm.tile([64, 64], F32R, tag="cattnT")
            nc.vector.tensor_copy(cattnT, ctp)

            pam_pair(4)
            # CAM apply + scbuf
            for (i0, iw) in IC:
                cam_ps = mcp.tile([64, iw], F32, tag="mc", name="cam_ps")
                nc.tensor.matmul(cam_ps, cattnT, feat[0:64, i0:i0 + iw],
                                 start=True, stop=True)
                tmpc = etp.tile([64, iw], F32R, tag="camt", bufs=3,
                                name="tmpc")
                nc.vector.tensor_scalar_mul(tmpc, cam_ps, gcam)
                r0, nr = i0 // W, iw // W
                nc.vector.tensor_tensor(
                    scbuf[0:64, r0:r0 + nr, 1:65],
                    tmpc[:, :].rearrange("p (r c) -> p r c", c=W),
                    feat[0:64, i0:i0 + iw].rearrange("p (r c) -> p r c", c=W),
                    ALU.add)
            nc.vector.tensor_scalar_mul(scbuf[0:64, 0, 1:65],
                                        scbuf[0:64, 0, 1:65], hmt[:, 0:1])
            nc.vector.tensor_scalar_mul(scbuf[0:64, 33, 1:65],
                                        scbuf[0:64, 33, 1:65], hmt[:, 1:2])
            for (a, b) in [(0, 9), (9, 17), (17, 25), (25, 33)]:
                nc.gpsimd.tensor_copy(scbuf[64:128, a:b, :],
                                      scbuf[0:64, a + 1:b + 1, :])

            def conv2(buf, y2sb, sttag):
                st = sm.tile([64, 4, 6], F32, tag=sttag, name=sttag)
                for T in range(4):
                    r0 = 1 + 8 * T
                    yps = mcp.tile([64, 512], F32, tag="mc", name="yps")
                    for dxi in range(3):
                        rhs1 = buf[:, r0 - 1:r0 + 7, dxi:dxi + 64]
                        nc.tensor.matmul(yps, w2at[:, dxi * 64:(dxi + 1) * 64],
                                         rhs1, start=(dxi == 0), stop=False)
                        rhs2 = buf[0:64, r0 + 1:r0 + 9, dxi:dxi + 64]
                        nc.tensor.matmul(yps, w2bt[:, dxi * 64:(dxi + 1) * 64],
                                         rhs2, start=False, stop=(dxi == 2))
                    nc.vector.bn_stats(st[:, T, :], yps)
                    nc.vector.tensor_copy(y2sb[:, T * 512:(T + 1) * 512], yps)
                mv = sm.tile([64, 2], F32, tag=sttag + "mv", name=sttag + "mv")
                nc.vector.bn_aggr(mv, st[:, :, :])
                return mv

            pam_pair(6)
            # conv2 on CAM branch + its stats AR (hidden under attention)
            mvb = conv2(scbuf, y2b, "stb")
            glb = stat_ar(mvb, "arb")
            scb, shb = bn_coeffs(glb, "bnb")
            rb = big.tile([64, MY], F32R, tag="rb")
            nc.scalar.activation(rb, y2b, AF.Relu, bias=shb, scale=scb)

            # ---- pam normalize (r = gamma_pam / s), sa = pam_u*r + feat1
            def pam_div(src, i0, iw, sfx):
                r32 = sm.tile([1, iw], F32, tag="r32", name="r32" + sfx)
                nc.vector.reciprocal(r32, src[64:65, :])
                rr = sm.tile([1, iw], F32R, tag="rr", name="rr" + sfx)
                nc.vector.tensor_scalar_mul(rr, r32, cst[0:1, 0:1])
                rbc = etp.tile([64, iw], F32R, tag="camt", bufs=3,
                               name="rbc" + sfx)
                nc.gpsimd.partition_broadcast(rbc, rr)
                tmpa = etp.tile([64, iw], F32R, tag="camt", bufs=3,
                                name="tmpa" + sfx)
                nc.vector.tensor_tensor(tmpa, src[0:64, :], rbc, ALU.mult)
                r0, nr = i0 // W, iw // W
                nc.vector.tensor_tensor(
                    sabuf[0:64, r0:r0 + nr, 1:65],
                    tmpa[:, :].rearrange("p (r c) -> p r c", c=W),
                    feat[0:64, i0:i0 + iw].rearrange("p (r c) -> p r c", c=W),
                    ALU.add)

            pam_pair(8, chunk_cb=lambda ici, i0, iw: pam_div(
                pacc[:, i0:i0 + iw], i0, iw, str(ici)))
            nc.vector.tensor_scalar_mul(sabuf[0:64, 0, 1:65],
                                        sabuf[0:64, 0, 1:65], hmt[:, 0:1])
            nc.vector.tensor_scalar_mul(sabuf[0:64, 33, 1:65],
                                        sabuf[0:64, 33, 1:65], hmt[:, 1:2])
            for (a, b) in [(0, 9), (9, 17), (17, 25), (25, 33)]:
                nc.gpsimd.tensor_copy(sabuf[64:128, a:b, :],
                                      sabuf[0:64, a + 1:b + 1, :])

            mva = conv2(sabuf, y2a, "sta")
            gla = stat_ar(mva, "ara")
            sca, sha = bn_coeffs(gla, "bna")

            # ---- relu + sum -> fp16 feat_sum; conv8 (1x1, 64->256) runs on
            # the host after download: 4x fewer D2H bytes over the tunnel.
            for T in range(4):
                sl = slice(T * 512, (T + 1) * 512)
                ra = etp.tile([64, 512], F32R, tag="camt", bufs=3,
                              name=f"ra{T}")
                nc.scalar.activation(ra, y2a[:, sl], AF.Relu,
                                     bias=sha, scale=sca)
                nc.vector.tensor_tensor(fs16[:, sl], ra, rb[:, sl], ALU.add)
                nc.sync.dma_start(out=out[:, sl], in_=fs16[:, sl])
    nc.finalize()
    return nc


# ---------------------------------------------------------------- runner
class _Runner:
    def __init__(self, nc, n_cores=NCORES):
        install_neuronx_cc_hook()
        self.nc = nc
        self.n_cores = n_cores
        in_names, out_names, out_avals, zero_shapes = [], [], [], []
        pname = nc.partition_id_tensor.name if nc.partition_id_tensor else None
        for alloc in nc.m.functions[0].allocations:
            if not isinstance(alloc, mybir.MemoryLocationSet):
                continue
            name = alloc.memorylocations[0].name
            if alloc.kind == "ExternalInput":
                if name != pname:
                    in_names.append(name)
            elif alloc.kind == "ExternalOutput":
                out_names.append(name)
                shape = tuple(alloc.tensor_shape)
                dtype = mybir.dt.np(alloc.dtype)
                out_avals.append(jax.core.ShapedArray(shape, dtype))
                zero_shapes.append((shape, dtype))
        self.n_params = len(in_names)
        self.in_names = in_names + out_names
        if pname is not None:
            self.in_names.append(pname)
        self.out_names = out_names

        devices = jax.devices()[:n_cores]
        self.mesh = Mesh(np.asarray(devices), ("core",))
        self.sharding = NamedSharding(self.mesh, PartitionSpec("core"))

        in_names_t = tuple(self.in_names)
        out_names_t = tuple(out_names)
        out_avals_t = tuple(out_avals)
        has_pid = pname is not None

        def _body(*args):
            operands = list(args)
            if has_pid:
                operands.append(partition_id_tensor())
            outs = _bass_exec_p.bind(
                *operands,
                out_avals=out_avals_t,
                in_names=in_names_t,
                out_names=out_names_t,
                lowering_input_output_aliases=(),
                sim_require_finite=True,
                sim_require_nnan=True,
                nc=nc,
            )
            return tuple(outs)

        n_args = self.n_params + len(out_names)
        self.fn = jax.jit(
            shard_map(_body, mesh=self.mesh,
                      in_specs=(PartitionSpec("core"),) * n_args,
                      out_specs=(PartitionSpec("core"),) * len(out_names),
                      check_rep=False),
            keep_unused=True,
        )
        # cached placeholder "output" operands: device-resident, never
        # donated, never transferred again. The kernel writes every output
        # element so their contents are irrelevant.
        self.placeholders = [
            jax.jit(lambda s=shape, d=dtype: jnp.zeros((n_cores * s[0],
                                                        *s[1:]), d),
                    out_shardings=self.sharding)()
            for shape, dtype in zero_shapes
        ]

    def __call__(self, *concat_inputs):
        dev_inputs = [jax.device_put(a, self.sharding) for a in concat_inputs]
        outs = self.fn(*dev_inputs, *self.placeholders)
        return [np.asarray(o) for o in outs]


_CACHE = {}


def kernel(**inputs):
    if "runner" not in _CACHE:
        _CACHE["runner"] = _Runner(_build())
    runner = _CACHE["runner"]
    ha, hb = _prep_core_inputs(
        np.asarray(inputs["x"], np.float32), np.asarray(inputs["w1"]),
        np.asarray(inputs["bn_g"]), np.asarray(inputs["bn_b"]),
        np.asarray(inputs["wq"]), np.asarray(inputs["bq"]),
        np.asarray(inputs["wk"]), np.asarray(inputs["bk"]),
        np.asarray(inputs["wv"]), np.asarray(inputs["bv"]),
        np.asarray(inputs["gamma_pam"]), np.asarray(inputs["gamma_cam"]),
        np.asarray(inputs["w2"]), np.asarray(inputs["w8"]),
        np.asarray(inputs["b8"]))
    res = runner(ha, hb)
    fs = res[0].reshape(NCORES, 64, MY).astype(np.float32)
    w8f = np.asarray(inputs["w8"], np.float32)[:, :, 0, 0]       # [256, 64]
    b8f = np.asarray(inputs["b8"], np.float32)[:, None]          # [256, 1]
    # conv8 = 1x1 conv: one GEMM over all cores' columns
    og = (w8f @ fs.transpose(1, 0, 2).reshape(64, NCORES * MY)) + b8f
    og = og.reshape(CO, NCORES, 32, W)
    out = np.empty((B, CO, H, W), np.float32)
    for c in range(NCORES):
        b, h = divmod(c, 2)
        out[b, :, 32 * h:32 * h + 32, :] = og[:, c]
    return out


# revision 12
# speedup vs baseline: 1.7039x; 1.0496x over previous
"""DANetHead Trainium2 kernel: 8-core SPMD (batch x row-half sharding).

Self-contained: hardcodes all shapes from the problem spec.

Per-core layout (core c: sample b=c//2, half h=c%2):
  P = [-1, 0..63, 64] (66 padded rows; -1/64 zero).
  feat local row L (0..65) holds padded row P[(L+32h) % 66] (cyclic rotation,
  so every core's attention/conv2 window is local rows 0..33 uniformly).
  window = local rows 0..33 (flat 0..2175); my output rows = 1..32.

Transfer-optimized: each core uploads only its own half of x (+1 halo row)
in fp16 inside a single packed blob; conv1 runs on the half, then the raw
conv1 outputs are pair-AllGathered on device and blended (per-core scalar
masks select the h=0/h=1 placement) into the full rotated feat layout.
Output is fp16. A custom PJRT runner avoids uploading donated zero output
buffers (the kernel writes every output element).
"""
import numpy as np

import jax
import jax.numpy as jnp
from jax.sharding import Mesh, PartitionSpec, NamedSharding
from jax.experimental.shard_map import shard_map

import concourse.bass as bass
import concourse.tile as tile
from concourse import bacc, mybir
from concourse.bass2jax import (_bass_exec_p, install_neuronx_cc_hook,
                                partition_id_tensor)

F32 = mybir.dt.float32
F32R = mybir.dt.float32r
F16 = mybir.dt.float16
AF = mybir.ActivationFunctionType
ALU = mybir.AluOpType

B, CIN, H, W = 4, 256, 64, 64
CI, CQ, CO = 64, 8, 256
NCORES = 8
LR = 66                  # local feat rows
NP = LR * W              # 4224
NJT = NP // 128          # 33 j-tiles
WIN = 34 * W             # 2176
MY = 32 * W              # 2048
NTAPS = 18               # 9 taps x 2 cin blocks
XR = 34                  # x rows per core (own 32 + halo)
# i chunks: CAM uses full window; PAM main loop uses ICM + tail
IC = [(0, 512), (512, 512), (1024, 512), (1536, 512), (2048, 128)]
ICM = [(0, 512), (512, 512), (1024, 512), (1536, 384), (1920, 256)]
N_STAT = 16384.0

# blob A (fp16) element offsets
OFF_XH = 0
OFF_W1 = OFF_XH + 128 * 2 * XR * 64      # 557056
OFF_W2A = OFF_W1 + 128 * NTAPS * CI      # 704512
OFF_W2B = OFF_W2A + 128 * 3 * CI         # 729088
OFF_QKV = OFF_W2B + 64 * 3 * CI          # 741376
OFF_ID = OFF_QKV + 65 * 80               # 746576
WT_END = OFF_ID + 128 * 128              # 762960
WT_LEN = WT_END - OFF_W1                 # 205904 shared fp16 elements
WT_CH = 128 * 202                        # 25856 per-core chunk (padded)
WT_PAD = NCORES * WT_CH - WT_LEN         # 944 zero-pad elements
# relative offsets inside the gathered weight tail
R_W1 = 0
R_W2A = 128 * NTAPS * CI                 # 147456
R_W2B = R_W2A + 128 * 3 * CI             # 172032
R_QKV = R_W2B + 64 * 3 * CI              # 184320
R_ID = R_QKV + 65 * 80                   # 189520
KA = OFF_W1 + WT_CH                      # 582794

# blob B (f32) element offsets
OFF_EB = 0
OFF_NM = OFF_EB + 2 * NP                 # 8448
OFF_HM = OFF_NM + 128 * NJT              # 12672
OFF_BG = OFF_HM + 64 * 2                 # 12800
OFF_CN = OFF_BG + 64 * 2                 # 12928
KB = OFF_CN + 2                          # 12930

PAD = [-1] + list(range(64)) + [64]


# ---------------------------------------------------------------- host prep
def _shared_blobs():
    """(blobA weight tail [KA-OFF_W1] fp16, blobB per-h variants [2, KB] f32)."""
    return None


def _prep_core_inputs(x, w1, bn_g, bn_b, wq, bq, wk, bk, wv, bv,
                      gamma_pam, gamma_cam, w2, w8, b8):
    f = np.float32
    # ---- shared weight tail of blob A (fp16)
    w1s = np.zeros((128, NTAPS, CI), f)
    for dy in range(3):
        for dx in range(3):
            for cb in range(2):
                s = (dy * 3 + dx) * 2 + cb
                w1s[:, s, :] = w1[:, cb * 128:(cb + 1) * 128, dy, dx].T
    w2a = np.zeros((128, 3, CI), f)
    w2b = np.zeros((64, 3, CI), f)
    for dx in range(3):
        w2a[:64, dx, :] = w2[:, :, 0, dx].T
        w2a[64:, dx, :] = w2[:, :, 1, dx].T
        w2b[:, dx, :] = w2[:, :, 2, dx].T
    wqkv = np.zeros((65, 80), f)
    wqkv[:64, 0:64] = wv[:, :, 0, 0].T
    wqkv[:64, 64:72] = wq[:, :, 0, 0].T
    wqkv[:64, 72:80] = wk[:, :, 0, 0].T
    wqkv[64, 0:64] = bv
    wqkv[64, 64:72] = bq
    wqkv[64, 72:80] = bk
    wtail = np.concatenate([
        w1s.ravel(), w2a.ravel(), w2b.ravel(), wqkv.ravel(),
        np.eye(128, dtype=f).ravel(),
        np.zeros(WT_PAD, f)]).astype(np.float16)

    # ---- blob B per-h variants (f32)
    bngb = np.stack([bn_g, bn_b], 1).astype(f)
    consts = np.array([float(gamma_pam[0]), float(gamma_cam[0])], f)
    hbv = np.zeros((2, KB), f)
    for h in range(2):
        centers = [PAD[(L + 32 * h) % 66] for L in range(LR)]
        realp = np.repeat(np.array([0 <= g <= 63 for g in centers]), W)
        ebias = np.concatenate([np.where(realp, 0.0, -1000.0).astype(f),
                                np.ones(NP, f)])
        nmask = np.where(realp, 1.0, 0.0).astype(f).reshape(NJT, 128).T
        hmask = np.zeros((64, 2), f)
        hmask[:, 0] = 0.0 if h == 0 else 1.0
        hmask[:, 1] = 0.0 if h == 1 else 1.0
        hbv[h] = np.concatenate([ebias, nmask.ravel(), hmask.ravel(),
                                 bngb.ravel(), consts])

    # ---- per-core blob A: xh [128, 2, 34, 64] fp16 + shared weight tail
    x16 = np.asarray(x, np.float16)
    ha = np.empty((NCORES, KA), np.float16)
    ha[:, OFF_W1:] = wtail.reshape(NCORES, WT_CH)
    hb = np.empty((NCORES, KB), f)
    for c in range(NCORES):
        b, h = divmod(c, 2)
        xh = np.zeros((128, 2, XR, 64), np.float16)
        if h == 0:
            # rows r=1..33 <- image rows 0..32 (r=0 is the zero pad row)
            xh[:, 0, 1:34, :] = x16[b, :128, 0:33, :]
            xh[:, 1, 1:34, :] = x16[b, 128:, 0:33, :]
        else:
            # rows r=0..32 <- image rows 31..63 (r=33 is the zero pad row)
            xh[:, 0, 0:33, :] = x16[b, :128, 31:64, :]
            xh[:, 1, 0:33, :] = x16[b, 128:, 31:64, :]
        ha[c, :OFF_W1] = xh.ravel()
        hb[c] = hbv[h]
    return ha, hb


# ---------------------------------------------------------------- bass build
def _build():
    nc = bacc.Bacc()
    ha = nc.declare_dram_parameter("ha", [1, KA], F16, isOutput=False)
    hb = nc.declare_dram_parameter("hb", [1, KB], F32R, isOutput=False)
    out = nc.declare_dram_parameter("out", [64, MY], F16, isOutput=True)

    def hap(off, ap):
        return bass.AP(tensor=ha, offset=off, ap=ap)

    def hbp(off, ap):
        return bass.AP(tensor=hb, offset=off, ap=ap)

    with tile.TileContext(nc) as tc:
        with tc.tile_pool(name="big", bufs=1) as big, \
             tc.tile_pool(name="wt", bufs=1) as wt, \
             tc.tile_pool(name="sm", bufs=2) as sm, \
             tc.tile_pool(name="et", bufs=2) as etp, \
             tc.tile_pool(name="ps", bufs=2, space="PSUM") as ps, \
             tc.tile_pool(name="pt", bufs=2, space="PSUM") as ptp, \
             tc.tile_pool(name="mc", bufs=2, space="PSUM") as mcp, \
             tc.tile_pool(name="dram", bufs=1, space="DRAM") as dram:

            # ---- persistent sbuf tensors
            feat = big.tile([65, NP], F32R, tag="feat")   # y1 then feat1(+ones)
            qkv = big.tile([80, NP], F32R, tag="qkv")
            qr = big.tile([128, WIN], F32R, tag="qr")
            kr4 = big.tile([128, 9, 128], F32R, tag="kr4")
            vT = big.tile([128, NJT, 65], F32R, tag="vT")
            fT = big.tile([128, NJT, CI], F32R, tag="fT")
            sabuf = big.tile([128, 34, LR], F32R, tag="sabuf")
            scbuf = big.tile([128, 34, LR], F32R, tag="scbuf")
            y2a = big.tile([64, MY], F32, tag="y2a")
            y2b = big.tile([64, MY], F32, tag="y2b")
            fs16 = big.tile([64, MY], F16, tag="fs16")
            pacc = big.tile([65, WIN], F32, tag="pacc")   # pam accumulator
            xh = big.tile([128, 2, XR, LR], F16, tag="xh")

            # ---- weights / consts in sbuf
            w1t = wt.tile([128, NTAPS, CI], F16, tag="w1t")
            wqkvt = wt.tile([65, 80], F32R, tag="wqkvt")
            w2at = wt.tile([128, 3 * CI], F32R, tag="w2at")
            w2bt = wt.tile([64, 3 * CI], F32R, tag="w2bt")
            bngbt = wt.tile([64, 2], F32, tag="bngbt")
            nmt = wt.tile([128, NJT], F32, tag="nmt")
            hmt = wt.tile([64, 2], F32, tag="hmt")
            cst = wt.tile([1, 2], F32, tag="cst")
            gcam = wt.tile([64, 1], F32, tag="gcam")
            epst = wt.tile([64, 1], F32, tag="epst")
            nc.vector.memset(epst, 1e-5)
            idt = wt.tile([128, 128], F32R, tag="idt")
            # fp16 staging for converted weights
            wq16 = wt.tile([65, 80], F16, tag="wq16")
            w2a16 = wt.tile([128, 3 * CI], F16, tag="w2a16")
            w2b16 = wt.tile([64, 3 * CI], F16, tag="w2b16")
            id16 = wt.tile([128, 128], F16, tag="id16")

            # reassemble the shared weight tail on device: each core ships
            # 1/8th, pair of DMAs bounce it to DRAM, all-8 AllGather restores
            # the full tail on every core.
            wcs = wt.tile([128, 202], F16, tag="wcs")
            nc.sync.dma_start(out=wcs, in_=hap(OFF_W1, [[202, 128], [1, 202]]))
            wcin = dram.tile([128, 202], F16, tag="wcin")
            nc.sync.dma_start(out=wcin[:, :], in_=wcs)
            wg = dram.tile([128 * NCORES, 202], F16, tag="wg")
            nc.gpsimd.collective_compute(
                "AllGather", ALU.bypass,
                replica_groups=[list(range(NCORES))],
                ins=[wcin.opt()], outs=[wg.opt()])

            def wgp(off, ap):
                return bass.AP(tensor=wg.tensor, offset=wg.offset + off, ap=ap)

            nc.sync.dma_start(out=w1t, in_=wgp(R_W1, [[NTAPS * CI, 128],
                                                      [CI, NTAPS], [1, CI]]))
            nc.sync.dma_start(out=w2a16, in_=wgp(R_W2A, [[192, 128], [1, 192]]))
            nc.sync.dma_start(out=w2b16, in_=wgp(R_W2B, [[192, 64], [1, 192]]))
            nc.sync.dma_start(out=wq16, in_=wgp(R_QKV, [[80, 65], [1, 80]]))
            nc.sync.dma_start(out=id16, in_=wgp(R_ID, [[128, 128], [1, 128]]))
            nc.vector.tensor_copy(w2at, w2a16)
            nc.vector.tensor_copy(w2bt, w2b16)
            nc.vector.tensor_copy(wqkvt, wq16)
            nc.vector.tensor_copy(idt, id16)

            nc.sync.dma_start(out=bngbt[:, :].bitcast(F32R), in_=hbp(OFF_BG, [[2, 64], [1, 2]]))
            nc.sync.dma_start(out=nmt[:, :].bitcast(F32R), in_=hbp(OFF_NM, [[NJT, 128], [1, NJT]]))
            nc.sync.dma_start(out=hmt[:, :].bitcast(F32R), in_=hbp(OFF_HM, [[2, 64], [1, 2]]))
            nc.sync.dma_start(out=cst[:, :].bitcast(F32R), in_=hbp(OFF_CN, [[2, 1], [1, 2]]))
            nc.gpsimd.dma_start(out=gcam, in_=hbp(OFF_CN + 1, [[0, 64], [1, 1]]))

            nc.gpsimd.memset(feat[64:65, :].bitcast(F32), 1.0)
            nc.gpsimd.memset(kr4[:, :, :].bitcast(F32), 0.0)
            nc.gpsimd.memset(vT[:, :, 64:65].bitcast(F32), 1.0)
            for bf in (sabuf, scbuf):
                nc.gpsimd.memset(bf[0:64, :, 0:1].bitcast(F32), 0.0)
                nc.gpsimd.memset(bf[0:64, :, 65:66].bitcast(F32), 0.0)

            # ---- x half: zero pad cols, DMA real cols
            nc.gpsimd.memset(xh[:, :, :, :], 0.0)
            nc.sync.dma_start(
                out=xh[:, :, :, 1:65],
                in_=hap(OFF_XH, [[2 * XR * 64, 128], [XR * 64, 2],
                                 [64, XR], [1, 64]]))

            # ---- conv1 own half -> feat cols 64:2112 (raw y1) + stats
            stats1 = sm.tile([64, 4, 6], F32, tag="stats1")
            for T in range(4):
                pst = mcp.tile([64, 512], F32, tag="mc", name=f"c1ps{T}")
                for s in range(NTAPS):
                    tap, cb = divmod(s, 2)
                    dy, dx = divmod(tap, 3)
                    rhs = xh[:, cb, 8 * T + dy:8 * T + dy + 8, dx:dx + 64]
                    nc.tensor.matmul(pst, w1t[:, s, :], rhs,
                                     start=(s == 0), stop=(s == NTAPS - 1))
                nc.vector.bn_stats(stats1[:, T, :], pst)
                nc.vector.tensor_copy(feat[0:64, 64 + 512 * T:576 + 512 * T], pst)
            mv1 = sm.tile([64, 2], F32, tag="mv1")
            nc.vector.bn_aggr(mv1, stats1[:, :, :])

            # ---- pair-exchange raw y1, blend partner rows into feat
            y1d = dram.tile([64, MY], F32, tag="y1d")
            y1g = dram.tile([128, MY], F32, tag="y1g")
            nc.sync.dma_start(out=y1d[:, :], in_=feat[0:64, 64:2112].bitcast(F32))
            nc.gpsimd.collective_compute(
                "AllGather", ALU.bypass,
                replica_groups=[[0, 1], [2, 3], [4, 5], [6, 7]],
                ins=[y1d.opt()], outs=[y1g.opt()])
            # stage X: placement for h=0 receivers; stage Y: for h=1 receivers.
            # stage col s maps to feat col 2112+s (s<2112) or s-2112 (s>=2112).
            stX = sm.tile([64, WIN], F32, tag="stX")
            stY = sm.tile([64, WIN], F32, tag="stY")
            nc.vector.memset(stX[:, 2048:2176], 0.0)
            nc.vector.memset(stY[:, 0:128], 0.0)
            nc.sync.dma_start(out=stX[:, 0:2048], in_=y1g[64:128, :])
            nc.sync.dma_start(out=stY[:, 128:2112], in_=y1g[0:64, 0:1984])
            nc.sync.dma_start(out=stY[:, 2112:2176], in_=y1g[0:64, 1984:2048])
            nc.vector.tensor_scalar_mul(stX, stX, hmt[:, 1:2])   # keep iff h==0
            nc.vector.tensor_scalar_mul(stY, stY, hmt[:, 0:1])   # keep iff h==1
            nc.vector.tensor_tensor(feat[0:64, 2112:4224],
                                    stX[:, 0:2112], stY[:, 0:2112], ALU.add)
            nc.vector.tensor_tensor(feat[0:64, 0:64],
                                    stX[:, 2112:2176], stY[:, 2112:2176],
                                    ALU.add)

            def bn_coeffs(gl, tag):
                """gl [64,2] = (sum, sumsq) -> (scale, shift) [64,1] f32."""
                mean = sm.tile([64, 1], F32, tag=tag + "m", name=tag + "m")
                var = sm.tile([64, 1], F32, tag=tag + "v", name=tag + "v")
                scl = sm.tile([64, 1], F32, tag=tag + "s", name=tag + "s")
                sh = sm.tile([64, 1], F32, tag=tag + "h", name=tag + "h")
                nc.vector.tensor_scalar_mul(mean, gl[:, 0:1], 1.0 / N_STAT)
                nc.vector.tensor_scalar_mul(var, gl[:, 1:2], 1.0 / N_STAT)
                nc.vector.tensor_tensor(scl, mean, mean, ALU.mult)
                nc.vector.tensor_tensor(var, var, scl, ALU.subtract)
                nc.scalar.activation(var, var, AF.Sqrt, bias=epst, scale=1.0)
                nc.vector.reciprocal(var, var)
                nc.vector.tensor_tensor(scl, bngbt[:, 0:1], var, ALU.mult)
                nc.vector.tensor_tensor(sh, mean, scl, ALU.mult)
                nc.vector.tensor_tensor(sh, bngbt[:, 1:2], sh, ALU.subtract)
                return scl, sh

            def stat_ar(mv, tag):
                """partial (mean,var over MY) -> AllReduce -> (sum,sumsq)."""
                ars = sm.tile([64, 2], F32, tag=tag + "s", name=tag + "s")
                t_t = sm.tile([64, 1], F32, tag=tag + "t", name=tag + "t")
                nc.vector.tensor_scalar_mul(ars[:, 0:1], mv[:, 0:1], float(MY))
                nc.vector.tensor_tensor(t_t, mv[:, 0:1], mv[:, 0:1], ALU.mult)
                nc.vector.tensor_tensor(t_t, mv[:, 1:2], t_t, ALU.add)
                nc.vector.tensor_scalar_mul(ars[:, 1:2], t_t, float(MY))
                a_in = dram.tile([64, 2], F32, tag=tag + "_in", name=tag + "_in")
                a_out = dram.tile([64, 2], F32, tag=tag + "_out",
                                  name=tag + "_out")
                nc.sync.dma_start(out=a_in[:, :], in_=ars)
                nc.gpsimd.collective_compute(
                    "AllReduce", ALU.add,
                    replica_groups=[list(range(NCORES))],
                    ins=[a_in.opt()], outs=[a_out.opt()])
                gl = sm.tile([64, 2], F32, tag=tag + "g", name=tag + "g")
                nc.sync.dma_start(out=gl, in_=a_out[:, :])
                return gl

            # AR1: bn1 stats -> relu(bn(y1)) over all 66 local rows
            gl1 = stat_ar(mv1, "ar1")
            sc1, sh1 = bn_coeffs(gl1, "bn1")
            for c0 in range(0, NP, 1056):
                sl = feat[0:64, c0:c0 + 1056]
                nc.scalar.activation(sl, sl, AF.Relu, bias=sh1, scale=sc1)

            # ---- qkv
            qkvtiles = [(t * 512, 512) for t in range(8)] + [(4096, 128)]
            for ti, (c0, cw) in enumerate(qkvtiles):
                qps = mcp.tile([80, cw], F32, tag="mc", name="qps")
                nc.tensor.matmul(qps, wqkvt, feat[:, c0:c0 + cw],
                                 start=True, stop=True)
                nc.vector.tensor_copy(qkv[:, c0:c0 + cw], qps)
            # qr: q replicated at partition groups; row 32g+8 = ones
            # (pairs with the ebias row in kr4 -> energy gets +ebias[j])
            for g in range(4):
                nc.sync.dma_start(out=qr[32 * g:32 * g + 8, :],
                                  in_=qkv[64:72, 0:WIN])
            for g in range(4):
                nc.sync.dma_start(
                    out=qr[32 * g + 8:32 * g + 9, :],
                    in_=hbp(OFF_EB + NP, [[NP, 1], [1, WIN]]))
            # kr4: k repartitioned per j-group; row 8 of each 32-block holds
            # the exp masking bias for that j-tile
            kbounce = dram.tile([8, NP], F32R, tag="kbounce", name="kbounce")
            nc.sync.dma_start(out=kbounce[:, :], in_=qkv[72:80, :])
            for u in range(4):
                ksrc = bass.AP(tensor=kbounce.tensor,
                               offset=kbounce.offset + u * 128,
                               ap=[[NP, 8], [512, 8], [1, 128]])
                nc.sync.dma_start(out=kr4[32 * u:32 * u + 8, 0:8, :], in_=ksrc)
                nc.sync.dma_start(
                    out=kr4[32 * u + 8:32 * u + 9, 0:8, :],
                    in_=hbp(OFF_EB + u * 128, [[512, 8], [1, 128]]))
            nc.sync.dma_start(out=kr4[0:8, 8, :], in_=kbounce[:, 4096:4224])
            nc.sync.dma_start(out=kr4[8:9, 8, :],
                              in_=hbp(OFF_EB + 4096, [[NP, 1], [1, 128]]))

            # ---- vT transpose (+ones col), 4 per psum bank
            for j0 in range(0, 32, 4):
                tp = mcp.tile([128, 4, 64], F32R, tag="mc", name=f"vtp{j0}")
                for k in range(4):
                    jt = j0 + k
                    nc.tensor.transpose(
                        tp[:, k, :],
                        qkv[0:64, jt * 128:(jt + 1) * 128],
                        idt[0:64, 0:64])
                nc.vector.tensor_copy(vT[:, j0:j0 + 4, 0:64], tp)
            tpl = mcp.tile([128, 64], F32R, tag="mc", name="vtpl")
            nc.tensor.transpose(tpl, qkv[0:64, 32 * 128:33 * 128],
                                idt[0:64, 0:64])
            nc.vector.tensor_copy(vT[:, 32, 0:64], tpl)

            # ================= interleaved attention + CAM emission ========
            def pam_pair(jg0, chunk_cb=None):
                """Emit energy/exp/pam for j-groups jg0, jg0+1 (or lone 8)."""
                jgs = [jg0] if jg0 == 8 else [jg0, jg0 + 1]
                for ici, (i0, iw) in enumerate(ICM):
                    pt = ptp.tile([65, iw], F32, tag="pt", name="pt")
                    nmm = sum(4 if j < 8 else 1 for j in jgs)
                    k = 0
                    for jg in jgs:
                        nu2 = 2 if jg < 8 else 1
                        for p in range(2 if jg < 8 else 1):
                            et_ps = ps.tile([128, 2, 512], F32, tag="ps",
                                            name="et_ps")
                            for u2 in range(nu2):
                                u = 2 * p + u2
                                nc.tensor.matmul(
                                    et_ps[:, u2, 0:iw],
                                    kr4[32 * u:32 * u + 32, jg, :],
                                    qr[32 * u:32 * u + 32, i0:i0 + iw],
                                    start=True, stop=True,
                                    tile_position=(32 * u, 0))
                            eT = etp.tile([128, 2, 512], F32R, tag="et",
                                          bufs=2, name="eT")
                            if nu2 == 2:
                                nc.scalar.activation(eT[:, :, 0:iw],
                                                     et_ps[:, :, 0:iw],
                                                     AF.Exp, bias=0.0,
                                                     scale=1.0)
                            else:
                                nc.scalar.activation(eT[:, 0, 0:iw],
                                                     et_ps[:, 0, 0:iw],
                                                     AF.Exp, bias=0.0,
                                                     scale=1.0)
                            for u2 in range(nu2):
                                jt = 4 * jg + 2 * p + u2
                                nc.tensor.matmul(pt, vT[:, jt, :],
                                                 eT[:, u2, 0:iw],
                                                 start=(k == 0),
                                                 stop=(k == nmm - 1))
                                k += 1
                    if jg0 == 0:
                        nc.vector.tensor_copy(pacc[:, i0:i0 + iw], pt)
                    else:
                        nc.vector.tensor_tensor(pacc[:, i0:i0 + iw],
                                                pacc[:, i0:i0 + iw], pt,
                                                ALU.add)
                    if chunk_cb is not None:
                        chunk_cb(ici, i0, iw)

            pam_pair(0)
            # fT transposes (CAM input), masked
            for jt in range(NJT):
                tp = mcp.tile([128, 64], F32R, tag="mc", name=f"ftp{jt}")
                nc.tensor.transpose(tp, feat[0:64, jt * 128:(jt + 1) * 128],
                                    idt[0:64, 0:64])
                nc.vector.tensor_scalar_mul(fT[:, jt, :], tp, nmt[:, jt:jt + 1])

            pam_pair(2)
            # CAM: ce (chunked), softmax, cattnT
            ce_sb = sm.tile([64, 64], F32, tag="ce_sb")
            for ci_, (j0, nj) in enumerate([(0, 9), (9, 8), (17, 8), (25, 8)]):
                ce_ps = mcp.tile([64, 64], F32, tag="mc", name=f"ce{ci_}")
                for k in range(nj):
                    jt = j0 + k
                    nc.tensor.matmul(ce_ps, fT[:, jt, :], fT[:, jt, :],
                                     start=(k == 0), stop=(k == nj - 1))
                if ci_ == 0:
                    nc.vector.tensor_copy(ce_sb, ce_ps)
                else:
                    nc.vector.tensor_tensor(ce_sb, ce_sb, ce_ps, ALU.add)
            rmin = sm.tile([64, 1], F32, tag="rmin")
            nc.vector.tensor_reduce(rmin, ce_sb, mybir.AxisListType.X, ALU.min)
            cu = sm.tile([64, 64], F32, tag="cu")
            nc.scalar.activation(cu, ce_sb, AF.Exp, bias=rmin, scale=-1.0)
            rs = sm.tile([64, 1], F32, tag="rs")
            nc.vector.tensor_reduce(rs, cu, mybir.AxisListType.X, ALU.add)
            nc.vector.reciprocal(rs, rs)
            cattn = sm.tile([64, 64], F32R, tag="cattn")
            nc.vector.tensor_scalar_mul(cattn, cu, rs)
            ctp = mcp.tile([64, 64], F32R, tag="mc", name="ctp")
            nc.tensor.transpose(ctp, cattn, idt[0:64, 0:64])
            cattnT = sm.tile([64, 64], F32R, tag="cattnT")
            nc.vector.tensor_copy(cattnT, ctp)

            pam_pair(4)
            # CAM apply + scbuf
            for (i0, iw) in IC:
                cam_ps = mcp.tile([64, iw], F32, tag="mc", name="cam_ps")
                nc.tensor.matmul(cam_ps, cattnT, feat[0:64, i0:i0 + iw],
                                 start=True, stop=True)
                tmpc = etp.tile([64, iw], F32R, tag="camt", bufs=3,
                                name="tmpc")
                nc.vector.tensor_scalar_mul(tmpc, cam_ps, gcam)
                r0, nr = i0 // W, iw // W
                nc.vector.tensor_tensor(
                    scbuf[0:64, r0:r0 + nr, 1:65],
                    tmpc[:, :].rearrange("p (r c) -> p r c", c=W),
                    feat[0:64, i0:i0 + iw].rearrange("p (r c) -> p r c", c=W),
                    ALU.add)
            nc.vector.tensor_scalar_mul(scbuf[0:64, 0, 1:65],
                                        scbuf[0:64, 0, 1:65], hmt[:, 0:1])
            nc.vector.tensor_scalar_mul(scbuf[0:64, 33, 1:65],
                                        scbuf[0:64, 33, 1:65], hmt[:, 1:2])
            for (a, b) in [(0, 9), (9, 17), (17, 25), (25, 33)]:
                nc.gpsimd.tensor_copy(scbuf[64:128, a:b, :],
                                      scbuf[0:64, a + 1:b + 1, :])

            def conv2(buf, y2sb, sttag):
                st = sm.tile([64, 4, 6], F32, tag=sttag, name=sttag)
                for T in range(4):
                    r0 = 1 + 8 * T
                    yps = mcp.tile([64, 512], F32, tag="mc", name="yps")
                    for dxi in range(3):
                        rhs1 = buf[:, r0 - 1:r0 + 7, dxi:dxi + 64]
                        nc.tensor.matmul(yps, w2at[:, dxi * 64:(dxi + 1) * 64],
                                         rhs1, start=(dxi == 0), stop=False)
                        rhs2 = buf[0:64, r0 + 1:r0 + 9, dxi:dxi + 64]
                        nc.tensor.matmul(yps, w2bt[:, dxi * 64:(dxi + 1) * 64],
                                         rhs2, start=False, stop=(dxi == 2))
                    nc.vector.bn_stats(st[:, T, :], yps)
                    nc.vector.tensor_copy(y2sb[:, T * 512:(T + 1) * 512], yps)
                mv = sm.tile([64, 2], F32, tag=sttag + "mv", name=sttag + "mv")
                nc.vector.bn_aggr(mv, st[:, :, :])
                return mv

            pam_pair(6)
            # conv2 on CAM branch + its stats AR (hidden under attention)
            mvb = conv2(scbuf, y2b, "stb")
            glb = stat_ar(mvb, "arb")
            scb, shb = bn_coeffs(glb, "bnb")
            rb = big.tile([64, MY], F32R, tag="rb")
            nc.scalar.activation(rb, y2b, AF.Relu, bias=shb, scale=scb)

            # ---- pam normalize (r = gamma_pam / s), sa = pam_u*r + feat1
            def pam_div(src, i0, iw, sfx):
                r32 = sm.tile([1, iw], F32, tag="r32", name="r32" + sfx)
                nc.vector.reciprocal(r32, src[64:65, :])
                rr = sm.tile([1, iw], F32R, tag="rr", name="rr" + sfx)
                nc.vector.tensor_scalar_mul(rr, r32, cst[0:1, 0:1])
                rbc = etp.tile([64, iw], F32R, tag="camt", bufs=3,
                               name="rbc" + sfx)
                nc.gpsimd.partition_broadcast(rbc, rr)
                tmpa = etp.tile([64, iw], F32R, tag="camt", bufs=3,
                                name="tmpa" + sfx)
                nc.vector.tensor_tensor(tmpa, src[0:64, :], rbc, ALU.mult)
                r0, nr = i0 // W, iw // W
                nc.vector.tensor_tensor(
                    sabuf[0:64, r0:r0 + nr, 1:65],
                    tmpa[:, :].rearrange("p (r c) -> p r c", c=W),
                    feat[0:64, i0:i0 + iw].rearrange("p (r c) -> p r c", c=W),
                    ALU.add)

            pam_pair(8, chunk_cb=lambda ici, i0, iw: pam_div(
                pacc[:, i0:i0 + iw], i0, iw, str(ici)))
            nc.vector.tensor_scalar_mul(sabuf[0:64, 0, 1:65],
                                        sabuf[0:64, 0, 1:65], hmt[:, 0:1])
            nc.vector.tensor_scalar_mul(sabuf[0:64, 33, 1:65],
                                        sabuf[0:64, 33, 1:65], hmt[:, 1:2])
            for (a, b) in [(0, 9), (9, 17), (17, 25), (25, 33)]:
                nc.gpsimd.tensor_copy(sabuf[64:128, a:b, :],
                                      sabuf[0:64, a + 1:b + 1, :])

            mva = conv2(sabuf, y2a, "sta")
            gla = stat_ar(mva, "ara")
            sca, sha = bn_coeffs(gla, "bna")

            # ---- relu + sum -> fp16 feat_sum; conv8 (1x1, 64->256) runs on
            # the host after download: 4x fewer D2H bytes over the tunnel.
            for T in range(4):
                sl = slice(T * 512, (T + 1) * 512)
                ra = etp.tile([64, 512], F32R, tag="camt", bufs=3,
                              name=f"ra{T}")
                nc.scalar.activation(ra, y2a[:, sl], AF.Relu,
                                     bias=sha, scale=sca)
                nc.vector.tensor_tensor(fs16[:, sl], ra, rb[:, sl], ALU.add)
                nc.sync.dma_start(out=out[:, sl], in_=fs16[:, sl])
    nc.finalize()
    return nc


# ---------------------------------------------------------------- runner
class _Runner:
    def __init__(self, nc, n_cores=NCORES):
        install_neuronx_cc_hook()
        self.nc = nc
        self.n_cores = n_cores
        in_names, out_names, out_avals, zero_shapes = [], [], [], []
        pname = nc.partition_id_tensor.name if nc.partition_id_tensor else None
        for alloc in nc.m.functions[0].allocations:
            if not isinstance(alloc, mybir.MemoryLocationSet):
                continue
            name = alloc.memorylocations[0].name
            if alloc.kind == "ExternalInput":
                if name != pname:
                    in_names.append(name)
            elif alloc.kind == "ExternalOutput":
                out_names.append(name)
                shape = tuple(alloc.tensor_shape)
                dtype = mybir.dt.np(alloc.dtype)
                out_avals.append(jax.core.ShapedArray(shape, dtype))
                zero_shapes.append((shape, dtype))
        self.n_params = len(in_names)
        self.in_names = in_names + out_names
        if pname is not None:
            self.in_names.append(pname)
        self.out_names = out_names

        devices = jax.devices()[:n_cores]
        self.mesh = Mesh(np.asarray(devices), ("core",))
        self.sharding = NamedSharding(self.mesh, PartitionSpec("core"))

        in_names_t = tuple(self.in_names)
        out_names_t = tuple(out_names)
        out_avals_t = tuple(out_avals)
        has_pid = pname is not None

        def _body(*args):
            operands = list(args)
            if has_pid:
                operands.append(partition_id_tensor())
            outs = _bass_exec_p.bind(
                *operands,
                out_avals=out_avals_t,
                in_names=in_names_t,
                out_names=out_names_t,
                lowering_input_output_aliases=(),
                sim_require_finite=True,
                sim_require_nnan=True,
                nc=nc,
            )
            return tuple(outs)

        n_args = self.n_params + len(out_names)
        self.fn = jax.jit(
            shard_map(_body, mesh=self.mesh,
                      in_specs=(PartitionSpec("core"),) * n_args,
                      out_specs=(PartitionSpec("core"),) * len(out_names),
                      check_rep=False),
            keep_unused=True,
        )
        # cached placeholder "output" operands: device-resident, never
        # donated, never transferred again. The kernel writes every output
        # element so their contents are irrelevant.
        self.placeholders = [
            jax.jit(lambda s=shape, d=dtype: jnp.zeros((n_cores * s[0],
                                                        *s[1:]), d),
                    out_shardings=self.sharding)()
            for shape, dtype in zero_shapes
        ]

    def __call__(self, *concat_inputs):
        dev_inputs = [jax.device_put(a, self.sharding) for a in concat_inputs]
        outs = self.fn(*dev_inputs, *self.placeholders)
        return [np.asarray(o) for o in outs]


_CACHE = {}


def kernel(**inputs):
    if "runner" not in _CACHE:
        _CACHE["runner"] = _Runner(_build())
    runner = _CACHE["runner"]
    ha, hb = _prep_core_inputs(
        np.asarray(inputs["x"], np.float32), np.asarray(inputs["w1"]),
        np.asarray(inputs["bn_g"]), np.asarray(inputs["bn_b"]),
        np.asarray(inputs["wq"]), np.asarray(inputs["bq"]),
        np.asarray(inputs["wk"]), np.asarray(inputs["bk"]),
        np.asarray(inputs["wv"]), np.asarray(inputs["bv"]),
        np.asarray(inputs["gamma_pam"]), np.asarray(inputs["gamma_cam"]),
        np.asarray(inputs["w2"]), np.asarray(inputs["w8"]),
        np.asarray(inputs["b8"]))
    res = runner(ha, hb)
    fs = res[0].reshape(NCORES, 64, MY).astype(np.float32)
    w8f = np.asarray(inputs["w8"], np.float32)[:, :, 0, 0]       # [256, 64]
    b8f = np.asarray(inputs["b8"], np.float32)[:, None]          # [256, 1]
    # conv8 = 1x1 conv: one GEMM over all cores' columns
    og = (w8f @ fs.transpose(1, 0, 2).reshape(64, NCORES * MY)) + b8f
    og = og.reshape(CO, NCORES, 32, W)
    out = np.empty((B, CO, H, W), np.float32)
    for c in range(NCORES):
        b, h = divmod(c, 2)
        out[b, :, 32 * h:32 * h + 32, :] = og[:, c]
    return out
